# revision 1
# baseline (speedup 1.0000x reference)
"""Trainium2 Bass kernel for nn_CrossAttention (B=8, Sq=Skv=2048, D=1024, C=768).

Strategy: data-parallel over batch — each of the 8 NeuronCores computes one
batch element's full cross-attention.

Per-core pipeline (all matmuls in float32r — TF32-like, 4x faster than fp32):
  phase 1a: K^T = Wk @ ctx^T (+bk) staged to DRAM;  V = ctx @ Wv^T (+bv) kept
            resident in SBUF [k, d].
  phase 1b: Q^T = (Wq @ x^T + bq)/sqrt(D) staged to DRAM in [d, q] layout.
  phase 2 (per 512-wide q block):
      scores^T[k,q] = KT_tile.T @ QT  (accumulate over d)        -> PSUM
      expT = exp(scores^T)            (ACT evacuation, no max — scores are
                                       bounded: |s| < ~3 for this problem)
      sums[1,q]  += ones.T @ expT     (PE matmul per k-tile)
      out^T[d,q]  = V_slice.T @ expT  (accumulate over k)
      final[q,o]  = outT_slice.T @ WoT (accumulate over d)
      final evac: * (1/sums[q]) (per-partition ACT scale) + bo, DMA out.

Softmax normalization commutes with the (linear) out-projection, so 1/sum is
applied on the final tiles where q sits on partitions.
"""

import numpy as np

import concourse.bass as bass  # noqa: F401  (bass types used via bacc/tile)
import concourse.mybir as mybir
import concourse.tile as tile
from concourse import bacc
from concourse.bass_utils import run_bass_kernel_spmd

# ---- problem shapes (hardcoded) ----
B, SQ, SKV, D, C = 8, 2048, 2048, 1024, 768
P = 128
DT = D // P          # 8  d-tiles
CT = C // P          # 6  c-tiles
KT = SKV // P        # 16 k-tiles
QB = 512             # q block width
NQB = SQ // QB       # 4 q blocks
KC = 512             # k chunk width in phase 1a
NKC = SKV // KC      # 4
SCALE = 1.0 / np.sqrt(np.float32(D))

F32 = mybir.dt.float32
F32R = mybir.dt.float32r
AF = mybir.ActivationFunctionType

_NC_CACHE = {}


def build():
    if "nc" in _NC_CACHE:
        return _NC_CACHE["nc"]
    nc = bacc.Bacc(trn_type="TRN2", num_swdge_queues=4)

    # ---- DRAM I/O (per-core slices; names = in_map keys) ----
    xT = nc.dram_tensor("xT", [D, SQ], F32R, kind="ExternalInput")
    ctxT = nc.dram_tensor("ctxT", [C, SKV], F32R, kind="ExternalInput")
    WqT = nc.dram_tensor("WqT", [D, D], F32R, kind="ExternalInput")
    WkT = nc.dram_tensor("WkT", [C, D], F32R, kind="ExternalInput")
    WvT = nc.dram_tensor("WvT", [C, D], F32R, kind="ExternalInput")
    WoT = nc.dram_tensor("WoT", [D, D], F32R, kind="ExternalInput")
    bqh = nc.dram_tensor("bqh", [P, DT], F32, kind="ExternalInput")  # bq*scale, [p, dt]
    bkh = nc.dram_tensor("bkh", [P, DT], F32, kind="ExternalInput")
    bvb = nc.dram_tensor("bvb", [P, D], F32, kind="ExternalInput")   # bv broadcast
    bob = nc.dram_tensor("bob", [P, D], F32, kind="ExternalInput")   # bo broadcast
    onesmat = nc.dram_tensor("onesmat", [P, P], F32R, kind="ExternalInput")  # all 1.0
    e0two = nc.dram_tensor("e0two", [P, 2], F32R, kind="ExternalInput")  # row0=1 else 0
    out = nc.dram_tensor("out", [SQ, D], F32, kind="ExternalOutput")

    with tile.TileContext(nc) as tc:
        with tc.tile_pool(name="persist", bufs=1) as persist, \
             tc.tile_pool(name="dstage", bufs=1, space="DRAM") as dstage:
            # intermediate stagings (DRAM pool tiles so Tile tracks the
            # staging-write -> reload-read dependency; raw dram_tensors are
            # not dep-tracked and the reloads would race the writes)
            KTst = dstage.tile([KT, DT, P, P], F32R, name="KTst")
            # one staging tile per q-block so phase 2's block-0 reload only
            # depends on block-0's writes (not all of phase 1b)
            QTst = [dstage.tile([DT, P, QB], F32R, name=f"QTst{qb}")
                    for qb in range(NQB)]
            v_sb = persist.tile([P, KT, D], F32R, name="v_sb")          # 64KB/p
            bq_sb = persist.tile([P, DT], F32, name="bq_sb")
            bk_sb = persist.tile([P, DT], F32, name="bk_sb")
            bv_sb = persist.tile([P, D], F32, name="bv_sb")
            bo_sb = persist.tile([P, D], F32, name="bo_sb")
            om_sb = persist.tile([P, P], F32R, name="om_sb")
            e0_sb = persist.tile([P, 2], F32R, name="e0_sb")
            sums_sb = persist.tile([P, QB], F32R, name="sums_sb")
            nc.sync.dma_start(bq_sb, bqh[:])
            nc.sync.dma_start(bk_sb, bkh[:])
            nc.sync.dma_start(bv_sb, bvb[:])
            nc.sync.dma_start(bo_sb, bob[:])
            nc.sync.dma_start(om_sb, onesmat[:])
            nc.sync.dma_start(e0_sb, e0two[:])

            # ================= phase 1a: K^T staging + V resident =========
            with tc.tile_pool(name="p1a_w", bufs=1) as p1a_w, \
                 tc.tile_pool(name="p1a_s", bufs=2) as p1a_s, \
                 tc.tile_pool(name="p1a_stg", bufs=4) as p1a_stg, \
                 tc.tile_pool(name="ps_k", bufs=2, space="PSUM") as ps_k, \
                 tc.tile_pool(name="ps_v", bufs=2, space="PSUM") as ps_v:
                wk_sb = p1a_w.tile([P, CT, D], F32R, name="wk_sb")
                wv_sb = p1a_w.tile([P, CT, D], F32R, name="wv_sb")
                # ~128KB DMA chunks (per-queue BW is only ~22GB/s) issued in
                # need-order so the first matmul group's operands land first
                ctx_tiles = []
                for kc in range(NKC):
                    ctx_tiles.append(
                        p1a_s.tile([P, CT, KC], F32R, name="ctx_sb", tag="ctx")
                        if kc < 2 else None)
                for t in range(CT):
                    nc.sync.dma_start(
                        ctx_tiles[0][:, t],
                        ctxT[t * P:(t + 1) * P, 0:KC])
                for quarter in range(4):
                    for t in range(CT):
                        nc.sync.dma_start(
                            wk_sb[:, t, quarter * 256:(quarter + 1) * 256],
                            WkT[t * P:(t + 1) * P, quarter * 256:(quarter + 1) * 256])
                for quarter in range(4):
                    for t in range(CT):
                        nc.sync.dma_start(
                            wv_sb[:, t, quarter * 256:(quarter + 1) * 256],
                            WvT[t * P:(t + 1) * P, quarter * 256:(quarter + 1) * 256])
                for t in range(CT):
                    nc.sync.dma_start(ctx_tiles[1][:, t],
                                      ctxT[t * P:(t + 1) * P, KC:2 * KC])
                for kc in range(NKC):
                    if ctx_tiles[kc] is None:
                        ctx_tiles[kc] = p1a_s.tile([P, CT, KC], F32R,
                                                   name="ctx_sb", tag="ctx")
                    ctx_sb = ctx_tiles[kc]
                    if kc > 1:
                        for t in range(CT):
                            nc.sync.dma_start(
                                ctx_sb[:, t],
                                ctxT[t * P:(t + 1) * P, kc * KC:(kc + 1) * KC])
                    # K^T tiles [d=128, k=512] for each d-tile
                    for dt_ in range(DT):
                        pk = ps_k.tile([P, KC], F32, name="pk", tag="pk")
                        for ct_ in range(CT):
                            nc.tensor.matmul(
                                pk, wk_sb[:, ct_, dt_ * P:(dt_ + 1) * P],
                                ctx_sb[:, ct_, :],
                                start=(ct_ == 0), stop=(ct_ == CT - 1))
                        kstg = p1a_stg.tile([P, KC], F32R, name="kstg", tag="kstg")
                        nc.scalar.activation(kstg, pk, AF.Identity,
                                             bias=bk_sb[:, dt_:dt_ + 1])
                        nc.gpsimd.dma_start(
                            KTst[4 * kc:4 * kc + 4, dt_].rearrange(
                                "t p i -> p t i"),
                            kstg.rearrange("p (t i) -> p t i", t=4))
                    # V tiles [k=128, d] resident
                    for t in range(4):
                        kt_ = kc * 4 + t
                        for dh in range(2):
                            pv = ps_v.tile([P, 512], F32, name="pv", tag="pv")
                            for ct_ in range(CT):
                                nc.tensor.matmul(
                                    pv, ctx_sb[:, ct_, t * P:(t + 1) * P],
                                    wv_sb[:, ct_, dh * 512:(dh + 1) * 512],
                                    start=(ct_ == 0), stop=(ct_ == CT - 1))
                            nc.vector.tensor_add(
                                v_sb[:, kt_, dh * 512:(dh + 1) * 512],
                                pv, bv_sb[:, dh * 512:(dh + 1) * 512])

            # wo pool spans 1b+2; its loads are issued mid-1b (after qb0's
            # gating loads) so phase 2 never waits on it
            with tc.tile_pool(name="p2_w", bufs=1) as p2_w, \
                 tc.tile_pool(name="p2_qt", bufs=2) as p2_qt:
                wo_sb = p2_w.tile([P, DT, D], F32R, name="wo_sb")
                qt0_sb = None

                # ================= phase 1b: Q^T staging ==================
                with tc.tile_pool(name="p1b_w", bufs=1) as p1b_w, \
                     tc.tile_pool(name="p1b_s", bufs=2) as p1b_s, \
                     tc.tile_pool(name="p1b_stg", bufs=2) as p1b_stg, \
                     tc.tile_pool(name="ps_q", bufs=2, space="PSUM") as ps_q:
                    wq_sb = p1b_w.tile([P, DT, D], F32R, name="wq_sb")
                    for t in range(DT):
                        nc.sync.dma_start(wq_sb[:, t, 0:256],
                                          WqT[t * P:(t + 1) * P, 0:256])
                    for qb in range(NQB):
                        xt_sb = p1b_s.tile([P, DT, QB], F32R, name="xt_sb",
                                           tag="xt")
                        for t in range(DT):
                            nc.sync.dma_start(
                                xt_sb[:, t],
                                xT[t * P:(t + 1) * P, qb * QB:(qb + 1) * QB])
                        if qb == 0:
                            for quarter in range(1, 4):
                                for t in range(DT):
                                    nc.sync.dma_start(
                                        wq_sb[:, t,
                                              quarter * 256:(quarter + 1) * 256],
                                        WqT[t * P:(t + 1) * P,
                                            quarter * 256:(quarter + 1) * 256])
                        if qb == 1:
                            # prefetch phase-2 block-0 QT (QTst[0] was
                            # finished during qb==0)
                            qt0_sb = p2_qt.tile([P, DT, QB], F32R,
                                                name="qt_sb", tag="qt")
                            for t in range(DT):
                                nc.sync.dma_start(qt0_sb[:, t], QTst[0][t])
                        for dt_ in range(DT):
                            pq = ps_q.tile([P, QB], F32, name="pq", tag="pq")
                            for it in range(DT):
                                nc.tensor.matmul(
                                    pq, wq_sb[:, it, dt_ * P:(dt_ + 1) * P],
                                    xt_sb[:, it, :],
                                    start=(it == 0), stop=(it == DT - 1))
                            qstg = p1b_stg.tile([P, QB], F32R, name="qstg",
                                                tag="qstg")
                            nc.scalar.activation(qstg, pq, AF.Identity,
                                                 bias=bq_sb[:, dt_:dt_ + 1],
                                                 scale=float(SCALE))
                            nc.gpsimd.dma_start(QTst[qb][dt_], qstg)

                # ============== phase 2: attention + out proj =============
                with tc.tile_pool(name="p2_kts", bufs=3) as p2_kts, \
                     tc.tile_pool(name="p2_big", bufs=1) as p2_big, \
                     tc.tile_pool(name="p2_fin", bufs=4) as p2_fin, \
                     tc.tile_pool(name="p2_rcp", bufs=2) as p2_rcp, \
                     tc.tile_pool(name="ps_sc", bufs=2, space="PSUM") as ps_sc, \
                     tc.tile_pool(name="ps_sum", bufs=1, space="PSUM") as ps_sum, \
                     tc.tile_pool(name="ps_rt", bufs=1, space="PSUM") as ps_rt, \
                     tc.tile_pool(name="ps_out", bufs=2, space="PSUM") as ps_out, \
                     tc.tile_pool(name="ps_fin", bufs=2, space="PSUM") as ps_fin:
                  for qb in range(NQB):
                    if qb == 0:
                        qt_sb = qt0_sb
                    else:
                        qt_sb = p2_qt.tile([P, DT, QB], F32R, name="qt_sb",
                                           tag="qt")
                        for t in range(DT):
                            nc.sync.dma_start(qt_sb[:, t], QTst[qb][t])
                    expt_sb = p2_big.tile([P, KT, QB], F32R, name="expt_sb",
                                          tag="expt")
                    psums = ps_sum.tile([P, QB], F32, name="psums", tag="psums")
                    # ---- scores^T + exp + denominator ----
                    for kt_ in range(KT):
                        kts = p2_kts.tile([P, DT, P], F32R, name="kts",
                                          tag="kts")
                        nc.sync.dma_start(
                            kts, KTst[kt_].rearrange("d p i -> p d i"))
                        psc = ps_sc.tile([P, QB], F32, name="psc", tag="psc")
                        for dt_ in range(DT):
                            nc.tensor.matmul(
                                psc, kts[:, dt_], qt_sb[:, dt_],
                                start=(dt_ == 0), stop=(dt_ == DT - 1))
                        nc.scalar.activation(expt_sb[:, kt_], psc, AF.Exp)
                        # every output partition gets the k-sum of expT
                        nc.tensor.matmul(
                            psums, om_sb, expt_sb[:, kt_],
                            start=(kt_ == 0), stop=(kt_ == KT - 1),
                            skip_group_check=True)
                    if qb == 0:
                        # wo streams in while qb0's PV runs; needed only by
                        # the final projection ~40us later
                        for quarter in range(4):
                            for t in range(DT):
                                nc.sync.dma_start(
                                    wo_sb[:, t,
                                          quarter * 256:(quarter + 1) * 256],
                                    WoT[t * P:(t + 1) * P,
                                        quarter * 256:(quarter + 1) * 256])
                    # ---- 1/sums, transposed to [q-on-partition, 1] ----
                    nc.scalar.copy(sums_sb, psums)
                    prt = ps_rt.tile([P, 8], F32, name="prt", tag="prt")
                    for qs in range(4):
                        nc.tensor.matmul(
                            prt[:, 2 * qs:2 * qs + 2],
                            sums_sb[:, qs * P:(qs + 1) * P], e0_sb,
                            start=True, stop=True)
                    recip = p2_rcp.tile([P, 8], F32, name="recip", tag="recip")
                    nc.vector.reciprocal(recip, prt)
                    # ---- out^T = V.T @ expT (d-quarter passes) ----
                    outt_sb = p2_big.tile([P, DT, QB], F32R, name="outt_sb",
                                          tag="outt")
                    for dp in range(4):
                        po0 = ps_out.tile([P, QB], F32, name="po0", tag="po")
                        po1 = ps_out.tile([P, QB], F32, name="po1", tag="po")
                        po = (po0, po1)
                        for kt_ in range(KT):
                            for dc in range(2):
                                d0 = dp * 256 + dc * P
                                nc.tensor.matmul(
                                    po[dc], v_sb[:, kt_, d0:d0 + P],
                                    expt_sb[:, kt_],
                                    start=(kt_ == 0), stop=(kt_ == KT - 1))
                        for dc in range(2):
                            nc.scalar.copy(outt_sb[:, dp * 2 + dc], po[dc])
                    # ---- final = out^T.T @ WoT, * 1/sums + bo ----
                    for qs in range(4):
                        for oc in range(2):
                            pf = ps_fin.tile([P, 512], F32, name="pf", tag="pf")
                            for dt_ in range(DT):
                                nc.tensor.matmul(
                                    pf, outt_sb[:, dt_, qs * P:(qs + 1) * P],
                                    wo_sb[:, dt_, oc * 512:(oc + 1) * 512],
                                    start=(dt_ == 0), stop=(dt_ == DT - 1))
                            fin = p2_fin.tile([P, 512], F32, name="fin",
                                              tag="fin")
                            nc.scalar.mul(fin, pf, recip[:, 2 * qs:2 * qs + 1])
                            nc.vector.tensor_add(fin, fin,
                                                 bo_sb[:, oc * 512:(oc + 1) * 512])
                            nc.sync.dma_start(
                                out[qb * QB + qs * P: qb * QB + (qs + 1) * P,
                                    oc * 512:(oc + 1) * 512], fin)
    nc.finalize()
    _NC_CACHE["nc"] = nc
    return nc


def _host_prep(x, context, Wq, bq, Wk, bk, Wv, bv, Wo, bo):
    """Build the 8 per-core input maps (host-side layout prep)."""
    x = np.asarray(x, dtype=np.float32)
    context = np.asarray(context, dtype=np.float32)
    WqT = np.ascontiguousarray(np.asarray(Wq, np.float32).T)   # [i, d]
    WkT = np.ascontiguousarray(np.asarray(Wk, np.float32).T)   # [c, d]
    WvT = np.ascontiguousarray(np.asarray(Wv, np.float32).T)   # [c, d]
    WoT = np.ascontiguousarray(np.asarray(Wo, np.float32).T)   # [d, o]
    scale = np.float32(1.0 / np.sqrt(np.float32(D)))
    bqh = np.ascontiguousarray(
        (np.asarray(bq, np.float32) * scale).reshape(DT, P).T)  # [p, dt]
    bkh = np.ascontiguousarray(np.asarray(bk, np.float32).reshape(DT, P).T)
    bvb = np.ascontiguousarray(
        np.broadcast_to(np.asarray(bv, np.float32)[None, :], (P, D)))
    bob = np.ascontiguousarray(
        np.broadcast_to(np.asarray(bo, np.float32)[None, :], (P, D)))
    onesmat = np.ones((P, P), np.float32)
    e0two = np.zeros((P, 2), np.float32)
    e0two[0, :] = 1.0
    shared = dict(WqT=WqT, WkT=WkT, WvT=WvT, WoT=WoT, bqh=bqh, bkh=bkh,
                  bvb=bvb, bob=bob, onesmat=onesmat, e0two=e0two)
    in_maps = []
    for b in range(B):
        m = dict(shared)
        m["xT"] = np.ascontiguousarray(x[b].T)        # [D, SQ]
        m["ctxT"] = np.ascontiguousarray(context[b].T)  # [C, SKV]
        in_maps.append(m)
    return in_maps


def kernel(**inputs) -> np.ndarray:
    nc = build()
    in_maps = _host_prep(**inputs)
    res = run_bass_kernel_spmd(nc, in_maps, core_ids=list(range(B)))
    return np.stack([res.results[b]["out"] for b in range(B)], axis=0)



# revision 2
# speedup vs baseline: 1.2735x; 1.2735x over previous
"""Trainium2 Bass kernel for nn_CrossAttention (B=8, Sq=Skv=2048, D=1024, C=768).

Strategy: data-parallel over batch — each of the 8 NeuronCores computes one
batch element's full cross-attention.

All matmul operands are bf16 (same PE row rate as fp32r on TRN2, half the
SBUF/DMA bytes), PSUM accumulation fp32. Everything stays SBUF-resident —
no DRAM staging roundtrips (the fp32r baseline moved ~95MB HBM/core; this
moves ~22MB).

Math simplifications (exact):
  - bk is dropped: scores include q·bk, constant over k for fixed q, which
    cancels between softmax numerator and denominator.
  - bv is folded into the output bias: att@1 = 1 after normalization, so
    out = (e@V0)/sums @ Wo^T + (bo + Wo@bv). Host precomputes bo'.

Per-core pipeline:
  phase 1a (per 512-wide k chunk): K^T[d,k] tiles and V[k,d] tiles from
    ctx chunk; both SBUF-resident bf16.
  phase 1b (per 512-wide q chunk): Q^T[d,q] = (Wq@x^T)*scale + bq*scale,
    SBUF-resident bf16.
  phase 2 (per 512-wide q block):
    scores^T[k,q] accumulated over d -> PSUM; exp via ACT -> expt bf16;
    denominator: DVE accumulates expt over the 16 k-tiles (fp32), then one
    fp32 ones-matmul reduces partitions + e0-trick transpose -> 1/sums per
    q-partition.
    out^T[d,q] = V_slice.T @ expT accumulated over k.
    final[q,o] = outT.T @ WoT; evac = (pf * recip + bo') in one DVE op.
"""

import numpy as np
import ml_dtypes

import concourse.bass as bass  # noqa: F401
import concourse.mybir as mybir
import concourse.tile as tile
from concourse import bacc
from concourse.bass_utils import run_bass_kernel_spmd

# ---- problem shapes (hardcoded) ----
B, SQ, SKV, D, C = 8, 2048, 2048, 1024, 768
P = 128
DT = D // P          # 8  d-tiles
CT = C // P          # 6  c-tiles
KT = SKV // P        # 16 k-tiles
QB = 512             # q block width
NQB = SQ // QB       # 4 q blocks
KC = 512             # k chunk width in phase 1a
NKC = SKV // KC      # 4
SCALE = 1.0 / np.sqrt(np.float32(D))

F32 = mybir.dt.float32
BF16 = mybir.dt.bfloat16
AF = mybir.ActivationFunctionType
ALU = mybir.AluOpType

_NC_CACHE = {}


def build():
    if "nc" in _NC_CACHE:
        return _NC_CACHE["nc"]
    nc = bacc.Bacc(trn_type="TRN2", num_swdge_queues=4)

    # ---- DRAM I/O (per-core slices; names = in_map keys) ----
    xT = nc.dram_tensor("xT", [D, SQ], BF16, kind="ExternalInput")
    ctxT = nc.dram_tensor("ctxT", [C, SKV], BF16, kind="ExternalInput")
    WqT = nc.dram_tensor("WqT", [D, D], BF16, kind="ExternalInput")
    WkT = nc.dram_tensor("WkT", [C, D], BF16, kind="ExternalInput")
    WvT = nc.dram_tensor("WvT", [C, D], BF16, kind="ExternalInput")
    WoT = nc.dram_tensor("WoT", [D, D], BF16, kind="ExternalInput")
    bqh = nc.dram_tensor("bqh", [P, DT], F32, kind="ExternalInput")   # bq*scale, [p, dt]
    bob = nc.dram_tensor("bob", [P, D], F32, kind="ExternalInput")    # (bo+Wo@bv) bcast
    onesmat = nc.dram_tensor("onesmat", [P, P], F32, kind="ExternalInput")
    e0two = nc.dram_tensor("e0two", [P, 2], F32, kind="ExternalInput")  # row0=1 else 0
    out = nc.dram_tensor("out", [SQ, D], F32, kind="ExternalOutput")

    with tile.TileContext(nc) as tc:
        with tc.tile_pool(name="persist", bufs=1) as persist:
            kt_sb = persist.tile([P, DT, SKV], BF16, name="kt_sb")    # 32KB/p
            v_sb = persist.tile([P, KT, D], BF16, name="v_sb")        # 32KB/p
            qt_sb = persist.tile([P, DT, SQ], BF16, name="qt_sb")     # 32KB/p
            wo_sb = persist.tile([P, DT, D], BF16, name="wo_sb")      # 16KB/p
            bq_sb = persist.tile([P, DT], F32, name="bq_sb")
            bo_sb = persist.tile([P, D], F32, name="bo_sb")
            om_sb = persist.tile([P, P], F32, name="om_sb")
            e0_sb = persist.tile([P, 2], F32, name="e0_sb")
            sums_sb = persist.tile([P, QB], F32, name="sums_sb")
            nc.sync.dma_start(bq_sb, bqh[:])
            nc.sync.dma_start(bo_sb, bob[:])
            nc.sync.dma_start(om_sb, onesmat[:])
            nc.sync.dma_start(e0_sb, e0two[:])

            # 1b/2 weight tiles are allocated up-front so their DMAs can be
            # issued mid-phase-1a (SBUF peak still fits)
            with tc.tile_pool(name="p1b_w", bufs=1) as p1b_w, \
                 tc.tile_pool(name="p1b_s", bufs=2) as p1b_s:
                wq_sb = p1b_w.tile([P, DT, D], BF16, name="wq_sb")

                # ============ phase 1a: K^T + V resident ============
                with tc.tile_pool(name="p1a_w", bufs=1) as p1a_w, \
                     tc.tile_pool(name="p1a_s", bufs=3) as p1a_s, \
                     tc.tile_pool(name="ps_k", bufs=2, space="PSUM") as ps_k, \
                     tc.tile_pool(name="ps_v", bufs=2, space="PSUM") as ps_v:
                    wk_sb = p1a_w.tile([P, CT, D], BF16, name="wk_sb")
                    wv_sb = p1a_w.tile([P, CT, D], BF16, name="wv_sb")
                    ctx_tiles = [p1a_s.tile([P, CT, KC], BF16, name="ctx_sb",
                                            tag="ctx") if kc < 3 else None
                                 for kc in range(NKC)]
                    # need-order DMA: chunk-0 ctx + wk first-half interleaved
                    # so the first accumulation group can start ASAP
                    for t in range(CT):
                        nc.sync.dma_start(ctx_tiles[0][:, t],
                                          ctxT[t * P:(t + 1) * P, 0:KC])
                        nc.sync.dma_start(wk_sb[:, t, 0:512],
                                          WkT[t * P:(t + 1) * P, 0:512])
                    for t in range(CT):
                        nc.sync.dma_start(wk_sb[:, t, 512:1024],
                                          WkT[t * P:(t + 1) * P, 512:1024])
                    for dh in range(2):
                        for t in range(CT):
                            nc.sync.dma_start(
                                wv_sb[:, t, dh * 512:(dh + 1) * 512],
                                WvT[t * P:(t + 1) * P, dh * 512:(dh + 1) * 512])
                    for t in range(CT):
                        nc.sync.dma_start(ctx_tiles[1][:, t],
                                          ctxT[t * P:(t + 1) * P, KC:2 * KC])
                    # 1b weights + first x chunk issued early (needed ~85us in)
                    for it in range(DT):
                        nc.sync.dma_start(wq_sb[:, it], WqT[it * P:(it + 1) * P, :])
                    xt0_sb = p1b_s.tile([P, DT, QB], BF16, name="xt_sb", tag="xt")
                    for it in range(DT):
                        nc.sync.dma_start(xt0_sb[:, it], xT[it * P:(it + 1) * P, 0:QB])
                    for t in range(CT):
                        nc.sync.dma_start(ctx_tiles[2][:, t],
                                          ctxT[t * P:(t + 1) * P, 2 * KC:3 * KC])

                    for kc in range(NKC):
                        if ctx_tiles[kc] is None:
                            ctx_tiles[kc] = p1a_s.tile([P, CT, KC], BF16,
                                                       name="ctx_sb", tag="ctx")
                            for t in range(CT):
                                nc.sync.dma_start(
                                    ctx_tiles[kc][:, t],
                                    ctxT[t * P:(t + 1) * P, kc * KC:(kc + 1) * KC])
                        ctx_sb = ctx_tiles[kc]
                        # K^T tiles [d=128, k=512] per d-tile
                        for dt_ in range(DT):
                            pk = ps_k.tile([P, KC], F32, name="pk", tag="pk")
                            for ct_ in range(CT):
                                nc.tensor.matmul(
                                    pk, wk_sb[:, ct_, dt_ * P:(dt_ + 1) * P],
                                    ctx_sb[:, ct_, :],
                                    start=(ct_ == 0), stop=(ct_ == CT - 1))
                            nc.scalar.copy(kt_sb[:, dt_, kc * KC:(kc + 1) * KC], pk)
                        # V tiles [k=128, d] resident
                        for t in range(4):
                            kt_ = kc * 4 + t
                            for dh in range(2):
                                pv = ps_v.tile([P, 512], F32, name="pv", tag="pv")
                                for ct_ in range(CT):
                                    nc.tensor.matmul(
                                        pv, ctx_sb[:, ct_, t * P:(t + 1) * P],
                                        wv_sb[:, ct_, dh * 512:(dh + 1) * 512],
                                        start=(ct_ == 0), stop=(ct_ == CT - 1))
                                nc.vector.tensor_scalar_mul(
                                    v_sb[:, kt_, dh * 512:(dh + 1) * 512], pv, 1.0)

                # ============ phase 1b: Q^T resident ============
                with tc.tile_pool(name="ps_q", bufs=2, space="PSUM") as ps_q:
                    for qc in range(NQB):
                        if qc == 0:
                            xt_sb = xt0_sb
                        else:
                            xt_sb = p1b_s.tile([P, DT, QB], BF16, name="xt_sb",
                                               tag="xt")
                            for it in range(DT):
                                nc.sync.dma_start(
                                    xt_sb[:, it],
                                    xT[it * P:(it + 1) * P, qc * QB:(qc + 1) * QB])
                        if qc == 1:
                            # wo needed ~60us into phase 2; stream it now
                            for it in range(DT):
                                nc.sync.dma_start(wo_sb[:, it],
                                                  WoT[it * P:(it + 1) * P, :])
                        for dt_ in range(DT):
                            pq = ps_q.tile([P, QB], F32, name="pq", tag="pq")
                            for it in range(DT):
                                nc.tensor.matmul(
                                    pq, wq_sb[:, it, dt_ * P:(dt_ + 1) * P],
                                    xt_sb[:, it, :],
                                    start=(it == 0), stop=(it == DT - 1))
                            nc.scalar.activation(
                                qt_sb[:, dt_, qc * QB:(qc + 1) * QB], pq,
                                AF.Identity, bias=bq_sb[:, dt_:dt_ + 1],
                                scale=float(SCALE))

            # ============ phase 2: attention + out proj ============
            with tc.tile_pool(name="p2_big", bufs=1) as p2_big, \
                 tc.tile_pool(name="p2_acc", bufs=2) as p2_acc, \
                 tc.tile_pool(name="p2_fin", bufs=4) as p2_fin, \
                 tc.tile_pool(name="p2_rcp", bufs=2) as p2_rcp, \
                 tc.tile_pool(name="ps_sc", bufs=2, space="PSUM") as ps_sc, \
                 tc.tile_pool(name="ps_sum", bufs=1, space="PSUM") as ps_sum, \
                 tc.tile_pool(name="ps_rt", bufs=1, space="PSUM") as ps_rt, \
                 tc.tile_pool(name="ps_po", bufs=2, space="PSUM") as ps_po, \
                 tc.tile_pool(name="ps_fin", bufs=2, space="PSUM") as ps_fin:
                expt_sb = p2_big.tile([P, KT, QB], BF16, name="expt_sb")
                outt_sb = p2_big.tile([P, DT, QB], BF16, name="outt_sb")
                for qb in range(NQB):
                    # ---- scores^T + exp; DVE accumulates denominator ----
                    acc = None
                    for kt_ in range(KT):
                        psc = ps_sc.tile([P, QB], F32, name="psc", tag="psc")
                        for dt_ in range(DT):
                            nc.tensor.matmul(
                                psc, kt_sb[:, dt_, kt_ * P:(kt_ + 1) * P],
                                qt_sb[:, dt_, qb * QB:(qb + 1) * QB],
                                start=(dt_ == 0), stop=(dt_ == DT - 1))
                        nc.scalar.activation(expt_sb[:, kt_], psc, AF.Exp)
                        nacc = p2_acc.tile([P, QB], F32, name="acc", tag="acc")
                        if kt_ == 0:
                            nc.vector.tensor_scalar_mul(nacc, expt_sb[:, 0], 1.0)
                        else:
                            nc.vector.tensor_add(nacc, acc, expt_sb[:, kt_])
                        acc = nacc
                    # ---- PV first quarter (keeps PE busy while sums settle) --
                    def pv_quarter(dp):
                        po0 = ps_po.tile([P, QB], F32, name="po0", tag="po")
                        po1 = ps_po.tile([P, QB], F32, name="po1", tag="po")
                        po = (po0, po1)
                        for kt_ in range(KT):
                            for dc in range(2):
                                d0 = dp * 256 + dc * P
                                nc.tensor.matmul(
                                    po[dc], v_sb[:, kt_, d0:d0 + P],
                                    expt_sb[:, kt_],
                                    start=(kt_ == 0), stop=(kt_ == KT - 1))
                        for dc in range(2):
                            nc.scalar.copy(outt_sb[:, dp * 2 + dc], po[dc])
                    pv_quarter(0)
                    # ---- sums: fp32 partition-reduce + e0-trick transpose ----
                    psums = ps_sum.tile([P, QB], F32, name="psums", tag="psums")
                    nc.tensor.matmul(psums, om_sb, acc, start=True, stop=True)
                    nc.scalar.copy(sums_sb, psums)
                    prt = ps_rt.tile([P, 8], F32, name="prt", tag="prt")
                    for qs in range(4):
                        nc.tensor.matmul(
                            prt[:, 2 * qs:2 * qs + 2],
                            sums_sb[:, qs * P:(qs + 1) * P], e0_sb,
                            start=True, stop=True)
                    recip = p2_rcp.tile([P, 8], F32, name="recip", tag="recip")
                    nc.vector.reciprocal(recip, prt)
                    for dp in range(1, 4):
                        pv_quarter(dp)
                    # ---- final = out^T.T @ WoT; evac fuses *recip + bo' ----
                    for qs in range(4):
                        for oc in range(2):
                            pf = ps_fin.tile([P, 512], F32, name="pf", tag="pf")
                            for dt_ in range(DT):
                                nc.tensor.matmul(
                                    pf, outt_sb[:, dt_, qs * P:(qs + 1) * P],
                                    wo_sb[:, dt_, oc * 512:(oc + 1) * 512],
                                    start=(dt_ == 0), stop=(dt_ == DT - 1))
                            fin = p2_fin.tile([P, 512], F32, name="fin",
                                              tag="fin")
                            nc.vector.scalar_tensor_tensor(
                                fin, pf, recip[:, 2 * qs:2 * qs + 1],
                                bo_sb[:, oc * 512:(oc + 1) * 512],
                                op0=ALU.mult, op1=ALU.add)
                            nc.sync.dma_start(
                                out[qb * QB + qs * P: qb * QB + (qs + 1) * P,
                                    oc * 512:(oc + 1) * 512], fin)
    nc.finalize()
    _NC_CACHE["nc"] = nc
    return nc


def _host_prep(x, context, Wq, bq, Wk, bk, Wv, bv, Wo, bo):
    """Build the 8 per-core input maps (host-side layout prep)."""
    BF = ml_dtypes.bfloat16
    x = np.asarray(x, dtype=np.float32)
    context = np.asarray(context, dtype=np.float32)
    WqT = np.ascontiguousarray(np.asarray(Wq, np.float32).T).astype(BF)   # [i, d]
    WkT = np.ascontiguousarray(np.asarray(Wk, np.float32).T).astype(BF)   # [c, d]
    WvT = np.ascontiguousarray(np.asarray(Wv, np.float32).T).astype(BF)   # [c, d]
    WoT = np.ascontiguousarray(np.asarray(Wo, np.float32).T).astype(BF)   # [d, o]
    scale = np.float32(1.0 / np.sqrt(np.float32(D)))
    bqh = np.ascontiguousarray(
        (np.asarray(bq, np.float32) * scale).reshape(DT, P).T)  # [p, dt]
    bo_eff = (np.asarray(bo, np.float64)
              + np.asarray(Wo, np.float64) @ np.asarray(bv, np.float64))
    bob = np.ascontiguousarray(
        np.broadcast_to(bo_eff.astype(np.float32)[None, :], (P, D)))
    onesmat = np.ones((P, P), np.float32)
    e0two = np.zeros((P, 2), np.float32)
    e0two[0, :] = 1.0
    shared = dict(WqT=WqT, WkT=WkT, WvT=WvT, WoT=WoT, bqh=bqh,
                  bob=bob, onesmat=onesmat, e0two=e0two)
    xbf = x.astype(BF)
    cbf = context.astype(BF)
    in_maps = []
    for b in range(B):
        m = dict(shared)
        m["xT"] = np.ascontiguousarray(xbf[b].T)        # [D, SQ]
        m["ctxT"] = np.ascontiguousarray(cbf[b].T)      # [C, SKV]
        in_maps.append(m)
    return in_maps


def kernel(**inputs) -> np.ndarray:
    nc = build()
    in_maps = _host_prep(**inputs)
    res = run_bass_kernel_spmd(nc, in_maps, core_ids=list(range(B)))
    return np.stack([res.results[b]["out"] for b in range(B)], axis=0)


# revision 11
# speedup vs baseline: 1.4333x; 1.1254x over previous
"""Trainium2 Bass kernel for nn_CrossAttention (B=8, Sq=Skv=2048, D=1024, C=768).

Strategy: data-parallel over batch — each of the 8 NeuronCores computes one
batch element's full cross-attention.

All matmul operands are bf16 (same PE row rate as fp32r on TRN2, half the
SBUF/DMA bytes), PSUM accumulation fp32. Everything stays SBUF-resident —
no DRAM staging roundtrips (the fp32r baseline moved ~95MB HBM/core; this
moves ~22MB).

Math simplifications (exact):
  - bk is dropped: scores include q·bk, constant over k for fixed q, which
    cancels between softmax numerator and denominator.
  - bv is folded into the output bias: att@1 = 1 after normalization, so
    out = (e@V0)/sums @ Wo^T + (bo + Wo@bv). Host precomputes bo'.

Per-core pipeline:
  phase 1a (per 512-wide k chunk): K^T[d,k] tiles and V[k,d] tiles from
    ctx chunk; both SBUF-resident bf16.
  phase 1b (per 512-wide q chunk): Q^T[d,q] = (Wq@x^T)*scale + bq*scale,
    SBUF-resident bf16.
  phase 2 (per 512-wide q block):
    scores^T[k,q] accumulated over d -> PSUM; exp via ACT -> expt bf16;
    denominator: DVE accumulates expt over the 16 k-tiles (fp32), then one
    fp32 ones-matmul reduces partitions + e0-trick transpose -> 1/sums per
    q-partition.
    out^T[d,q] = V_slice.T @ expT accumulated over k.
    final[q,o] = outT.T @ WoT; evac = (pf * recip + bo') in one DVE op.
"""

import numpy as np
import ml_dtypes

import concourse.bass as bass  # noqa: F401
import concourse.mybir as mybir
import concourse.tile as tile
from concourse import bacc
from concourse.bass_utils import run_bass_kernel_spmd

# ---- problem shapes (hardcoded) ----
B, SQ, SKV, D, C = 8, 2048, 2048, 1024, 768
P = 128
DT = D // P          # 8  d-tiles
CT = C // P          # 6  c-tiles
KT = SKV // P        # 16 k-tiles
QB = 512             # q block width
NQB = SQ // QB       # 4 q blocks
KC = 512             # k chunk width in phase 1a
NKC = SKV // KC      # 4
SCALE = 1.0 / np.sqrt(np.float32(D))

F32 = mybir.dt.float32
BF16 = mybir.dt.bfloat16
FP8 = mybir.dt.float8e4
AF = mybir.ActivationFunctionType
ALU = mybir.AluOpType
DR = mybir.MatmulPerfMode.DoubleRow

# scores (Q@K^T) in fp8e4m3 DoubleRow: 2x PE throughput on 26% of the FLOPs.
# K/Q are stored unscaled (sigma ~0.5, comfortably in e4m3 normal range);
# the 1/sqrt(D) score scale moves into the Exp activation's scale operand.
SCORES_FP8 = True

_NC_CACHE = {}


def build():
    if "nc" in _NC_CACHE:
        return _NC_CACHE["nc"]
    nc = bacc.Bacc(trn_type="TRN2", num_swdge_queues=4)

    # ---- DRAM I/O (per-core slices; names = in_map keys) ----
    xT = nc.dram_tensor("xT", [D, SQ], BF16, kind="ExternalInput")
    ctxT = nc.dram_tensor("ctxT", [C, SKV], BF16, kind="ExternalInput")
    WqT = nc.dram_tensor("WqT", [D, D], BF16, kind="ExternalInput")
    WkT = nc.dram_tensor("WkT", [C, D], BF16, kind="ExternalInput")
    WvT = nc.dram_tensor("WvT", [C, D], BF16, kind="ExternalInput")
    WoT = nc.dram_tensor("WoT", [D, D], BF16, kind="ExternalInput")
    bqh = nc.dram_tensor("bqh", [P, DT], F32, kind="ExternalInput")   # bq*scale, [p, dt]
    bob = nc.dram_tensor("bob", [P, D], F32, kind="ExternalInput")    # (bo+Wo@bv) bcast
    onesmat = nc.dram_tensor("onesmat", [P, P], F32, kind="ExternalInput")
    e0two = nc.dram_tensor("e0two", [P, 2], F32, kind="ExternalInput")  # row0=1 else 0
    out = nc.dram_tensor("out", [SQ, D], F32, kind="ExternalOutput")

    KQDT = FP8 if SCORES_FP8 else BF16
    with tile.TileContext(nc) as tc:
        with tc.tile_pool(name="persist", bufs=1) as persist:
            kt_sb = persist.tile([P, DT, SKV], KQDT, name="kt_sb")
            v_sb = persist.tile([P, KT, D], BF16, name="v_sb")        # 32KB/p
            qt_sb = persist.tile([P, DT, SQ], KQDT, name="qt_sb")
            wo_sb = persist.tile([P, DT, D], BF16, name="wo_sb")      # 16KB/p
            bq_sb = persist.tile([P, DT], F32, name="bq_sb")
            bo_sb = persist.tile([P, D], F32, name="bo_sb")
            om_sb = persist.tile([P, P], F32, name="om_sb")
            e0_sb = persist.tile([P, 2], F32, name="e0_sb")
            sums_sb = persist.tile([P, QB], F32, name="sums_sb")

            # 1b/2 weight tiles are allocated up-front so their DMAs can be
            # issued mid-phase-1a (SBUF peak still fits)
            with tc.tile_pool(name="p1b_w", bufs=1) as p1b_w, \
                 tc.tile_pool(name="p1b_s", bufs=2) as p1b_s:
                wq_sb = p1b_w.tile([P, DT, D], BF16, name="wq_sb")

                # ============ phase 1a: K^T + V resident ============
                with tc.tile_pool(name="p1a_w", bufs=1) as p1a_w, \
                     tc.tile_pool(name="p1a_s", bufs=3) as p1a_s, \
                     tc.tile_pool(name="ps_k", bufs=2, space="PSUM") as ps_k, \
                     tc.tile_pool(name="ps_v", bufs=2, space="PSUM") as ps_v:
                    wk_sb = p1a_w.tile([P, CT, D], BF16, name="wk_sb")
                    wv_sb = p1a_w.tile([P, CT, D], BF16, name="wv_sb")
                    ctx_tiles = [p1a_s.tile([P, CT, KC], BF16, name="ctx_sb",
                                            tag="ctx") if kc < 3 else None
                                 for kc in range(NKC)]
                    # need-order DMA, split across sync+gpsimd issue engines:
                    # the first accumulation group's operands go first
                    for t in range(CT):
                        nc.sync.dma_start(ctx_tiles[0][:, t],
                                          ctxT[t * P:(t + 1) * P, 0:KC])
                        nc.sync.dma_start(wk_sb[:, t, 0:512],
                                          WkT[t * P:(t + 1) * P, 0:512])
                    # gpsimd issues the second K-group half + V weights in
                    # parallel with sync's stream above
                    for t in range(CT):
                        nc.gpsimd.dma_start(wk_sb[:, t, 512:1024],
                                            WkT[t * P:(t + 1) * P, 512:1024])
                    for dh in range(2):
                        for t in range(CT):
                            nc.gpsimd.dma_start(
                                wv_sb[:, t, dh * 512:(dh + 1) * 512],
                                WvT[t * P:(t + 1) * P, dh * 512:(dh + 1) * 512])
                    nc.sync.dma_start(bq_sb, bqh[:])
                    nc.sync.dma_start(bo_sb, bob[:])
                    nc.sync.dma_start(om_sb, onesmat[:])
                    nc.sync.dma_start(e0_sb, e0two[:])
                    for t in range(CT):
                        nc.sync.dma_start(ctx_tiles[1][:, t],
                                          ctxT[t * P:(t + 1) * P, KC:2 * KC])
                    # 1b weights + first x chunk issued early (needed ~85us in)
                    for it in range(DT):
                        nc.gpsimd.dma_start(wq_sb[:, it],
                                            WqT[it * P:(it + 1) * P, :])
                    xt0_sb = p1b_s.tile([P, DT, QB], BF16, name="xt_sb", tag="xt")
                    for it in range(DT):
                        nc.sync.dma_start(xt0_sb[:, it], xT[it * P:(it + 1) * P, 0:QB])
                    for t in range(CT):
                        nc.sync.dma_start(ctx_tiles[2][:, t],
                                          ctxT[t * P:(t + 1) * P, 2 * KC:3 * KC])

                    for kc in range(NKC):
                        if ctx_tiles[kc] is None:
                            ctx_tiles[kc] = p1a_s.tile([P, CT, KC], BF16,
                                                       name="ctx_sb", tag="ctx")
                            for t in range(CT):
                                nc.sync.dma_start(
                                    ctx_tiles[kc][:, t],
                                    ctxT[t * P:(t + 1) * P, kc * KC:(kc + 1) * KC])
                        ctx_sb = ctx_tiles[kc]
                        # K^T tiles [d=128, k=512] per d-tile
                        for dt_ in range(DT):
                            pk = ps_k.tile([P, KC], F32, name="pk", tag="pk")
                            for ct_ in range(CT):
                                nc.tensor.matmul(
                                    pk, wk_sb[:, ct_, dt_ * P:(dt_ + 1) * P],
                                    ctx_sb[:, ct_, :],
                                    start=(ct_ == 0), stop=(ct_ == CT - 1))
                            nc.scalar.copy(kt_sb[:, dt_, kc * KC:(kc + 1) * KC], pk)
                        # V tiles [k=128, d] resident
                        for t in range(4):
                            kt_ = kc * 4 + t
                            for dh in range(2):
                                pv = ps_v.tile([P, 512], F32, name="pv", tag="pv")
                                for ct_ in range(CT):
                                    nc.tensor.matmul(
                                        pv, ctx_sb[:, ct_, t * P:(t + 1) * P],
                                        wv_sb[:, ct_, dh * 512:(dh + 1) * 512],
                                        start=(ct_ == 0), stop=(ct_ == CT - 1))
                                nc.vector.tensor_scalar_mul(
                                    v_sb[:, kt_, dh * 512:(dh + 1) * 512], pv, 1.0)

                # ============ phase 1b: Q^T resident ============
                with tc.tile_pool(name="ps_q", bufs=2, space="PSUM") as ps_q:
                    for qc in range(NQB):
                        if qc == 0:
                            xt_sb = xt0_sb
                        else:
                            xt_sb = p1b_s.tile([P, DT, QB], BF16, name="xt_sb",
                                               tag="xt")
                            for it in range(DT):
                                nc.sync.dma_start(
                                    xt_sb[:, it],
                                    xT[it * P:(it + 1) * P, qc * QB:(qc + 1) * QB])
                        if qc == 1:
                            # wo needed ~60us into phase 2; stream it now
                            for it in range(DT):
                                nc.gpsimd.dma_start(wo_sb[:, it],
                                                    WoT[it * P:(it + 1) * P, :])
                        for dt_ in range(DT):
                            pq = ps_q.tile([P, QB], F32, name="pq", tag="pq")
                            for it in range(DT):
                                nc.tensor.matmul(
                                    pq, wq_sb[:, it, dt_ * P:(dt_ + 1) * P],
                                    xt_sb[:, it, :],
                                    start=(it == 0), stop=(it == DT - 1))
                            # fp8 path: Q stays unscaled (the 1/sqrt(D) moves
                            # into Exp) to keep values in e4m3 normal range
                            nc.scalar.activation(
                                qt_sb[:, dt_, qc * QB:(qc + 1) * QB], pq,
                                AF.Identity, bias=bq_sb[:, dt_:dt_ + 1],
                                scale=1.0 if SCORES_FP8 else float(SCALE))

            # ============ phase 2: attention + out proj ============
            with tc.tile_pool(name="p2_big", bufs=1) as p2_big, \
                 tc.tile_pool(name="p2_acc", bufs=2) as p2_acc, \
                 tc.tile_pool(name="p2_fin", bufs=4) as p2_fin, \
                 tc.tile_pool(name="p2_rcp", bufs=2) as p2_rcp, \
                 tc.tile_pool(name="ps_sc", bufs=2, space="PSUM") as ps_sc, \
                 tc.tile_pool(name="ps_sum", bufs=1, space="PSUM") as ps_sum, \
                 tc.tile_pool(name="ps_rt", bufs=1, space="PSUM") as ps_rt, \
                 tc.tile_pool(name="ps_po", bufs=2, space="PSUM") as ps_po, \
                 tc.tile_pool(name="ps_fin", bufs=2, space="PSUM") as ps_fin:
                expt_sb = p2_big.tile([P, KT, QB], BF16, name="expt_sb")
                outt_sb = p2_big.tile([P, DT, QB], BF16, name="outt_sb")
                for qb in range(NQB):
                    # ---- scores^T + exp; DVE accumulates denominator ----
                    acc = None
                    for kt_ in range(KT):
                        psc = ps_sc.tile([P, QB], F32, name="psc", tag="psc")
                        if SCORES_FP8:
                            for dt_ in range(0, DT, 2):
                                nc.tensor.matmul(
                                    psc,
                                    kt_sb[:, dt_:dt_ + 2, kt_ * P:(kt_ + 1) * P],
                                    qt_sb[:, dt_:dt_ + 2, qb * QB:(qb + 1) * QB],
                                    start=(dt_ == 0), stop=(dt_ == DT - 2),
                                    perf_mode=DR)
                        else:
                            for dt_ in range(DT):
                                nc.tensor.matmul(
                                    psc, kt_sb[:, dt_, kt_ * P:(kt_ + 1) * P],
                                    qt_sb[:, dt_, qb * QB:(qb + 1) * QB],
                                    start=(dt_ == 0), stop=(dt_ == DT - 1))
                        nc.scalar.activation(
                            expt_sb[:, kt_], psc, AF.Exp,
                            scale=float(SCALE) if SCORES_FP8 else 1.0)
                        nacc = p2_acc.tile([P, QB], F32, name="acc", tag="acc")
                        if kt_ == 0:
                            nc.vector.tensor_scalar_mul(nacc, expt_sb[:, 0], 1.0)
                        else:
                            nc.vector.tensor_add(nacc, acc, expt_sb[:, kt_])
                        acc = nacc
                    # ---- PV first quarter (keeps PE busy while sums settle) --
                    def pv_quarter(dp):
                        po0 = ps_po.tile([P, QB], F32, name="po0", tag="po")
                        po1 = ps_po.tile([P, QB], F32, name="po1", tag="po")
                        po = (po0, po1)
                        for kt_ in range(KT):
                            for dc in range(2):
                                d0 = dp * 256 + dc * P
                                nc.tensor.matmul(
                                    po[dc], v_sb[:, kt_, d0:d0 + P],
                                    expt_sb[:, kt_],
                                    start=(kt_ == 0), stop=(kt_ == KT - 1))
                        for dc in range(2):
                            nc.scalar.copy(outt_sb[:, dp * 2 + dc], po[dc])
                    pv_quarter(0)
                    # ---- sums: fp32 partition-reduce + e0-trick transpose ----
                    psums = ps_sum.tile([P, QB], F32, name="psums", tag="psums")
                    nc.tensor.matmul(psums, om_sb, acc, start=True, stop=True)
                    nc.scalar.copy(sums_sb, psums)
                    prt = ps_rt.tile([P, 8], F32, name="prt", tag="prt")
                    for qs in range(4):
                        nc.tensor.matmul(
                            prt[:, 2 * qs:2 * qs + 2],
                            sums_sb[:, qs * P:(qs + 1) * P], e0_sb,
                            start=True, stop=True)
                    recip = p2_rcp.tile([P, 8], F32, name="recip", tag="recip")
                    nc.vector.reciprocal(recip, prt)
                    for dp in range(1, 4):
                        pv_quarter(dp)
                    # ---- final = out^T.T @ WoT; evac fuses *recip + bo' ----
                    for qs in range(4):
                        for oc in range(2):
                            pf = ps_fin.tile([P, 512], F32, name="pf", tag="pf")
                            for dt_ in range(DT):
                                nc.tensor.matmul(
                                    pf, outt_sb[:, dt_, qs * P:(qs + 1) * P],
                                    wo_sb[:, dt_, oc * 512:(oc + 1) * 512],
                                    start=(dt_ == 0), stop=(dt_ == DT - 1))
                            fin = p2_fin.tile([P, 512], F32, name="fin",
                                              tag="fin")
                            nc.vector.scalar_tensor_tensor(
                                fin, pf, recip[:, 2 * qs:2 * qs + 1],
                                bo_sb[:, oc * 512:(oc + 1) * 512],
                                op0=ALU.mult, op1=ALU.add)
                            nc.gpsimd.dma_start(
                                out[qb * QB + qs * P: qb * QB + (qs + 1) * P,
                                    oc * 512:(oc + 1) * 512], fin)
    nc.finalize()
    _NC_CACHE["nc"] = nc
    return nc


def _host_prep(x, context, Wq, bq, Wk, bk, Wv, bv, Wo, bo):
    """Build the 8 per-core input maps (host-side layout prep)."""
    BF = ml_dtypes.bfloat16
    x = np.asarray(x, dtype=np.float32)
    context = np.asarray(context, dtype=np.float32)
    WqT = np.ascontiguousarray(np.asarray(Wq, np.float32).T).astype(BF)   # [i, d]
    WkT = np.ascontiguousarray(np.asarray(Wk, np.float32).T).astype(BF)   # [c, d]
    WvT = np.ascontiguousarray(np.asarray(Wv, np.float32).T).astype(BF)   # [c, d]
    WoT = np.ascontiguousarray(np.asarray(Wo, np.float32).T).astype(BF)   # [d, o]
    scale = np.float32(1.0) if SCORES_FP8 else np.float32(1.0 / np.sqrt(np.float32(D)))
    bqh = np.ascontiguousarray(
        (np.asarray(bq, np.float32) * scale).reshape(DT, P).T)  # [p, dt]
    bo_eff = (np.asarray(bo, np.float64)
              + np.asarray(Wo, np.float64) @ np.asarray(bv, np.float64))
    bob = np.ascontiguousarray(
        np.broadcast_to(bo_eff.astype(np.float32)[None, :], (P, D)))
    onesmat = np.ones((P, P), np.float32)
    e0two = np.zeros((P, 2), np.float32)
    e0two[0, :] = 1.0
    shared = dict(WqT=WqT, WkT=WkT, WvT=WvT, WoT=WoT, bqh=bqh,
                  bob=bob, onesmat=onesmat, e0two=e0two)
    xbf = x.astype(BF)
    cbf = context.astype(BF)
    in_maps = []
    for b in range(B):
        m = dict(shared)
        m["xT"] = np.ascontiguousarray(xbf[b].T)        # [D, SQ]
        m["ctxT"] = np.ascontiguousarray(cbf[b].T)      # [C, SKV]
        in_maps.append(m)
    return in_maps


def kernel(**inputs) -> np.ndarray:
    nc = build()
    in_maps = _host_prep(**inputs)
    res = run_bass_kernel_spmd(nc, in_maps, core_ids=list(range(B)))
    return np.stack([res.results[b]["out"] for b in range(B)], axis=0)


# revision 16
# speedup vs baseline: 2.0910x; 1.4589x over previous
"""Trainium2 Bass kernel for nn_CrossAttention (B=8, Sq=Skv=2048, D=1024, C=768).

Strategy: data-parallel over batch — each of the 8 NeuronCores computes one
batch element's full cross-attention.

The projection chain is reassociated so every big contraction runs against
the NARROW context dim (C=768) instead of D=1024, and the K/V projections
disappear entirely (all exact identities, weights folded on host):

  scores = (x @ M + bqk) @ ctx^T          M   = Wq^T @ Wk   [D, C]
                                          bqk = bq @ Wk     [C]
  (bk drops: its score term is constant over k -> cancels in softmax)
  att    = softmax(scores / sqrt(D))
  final  = (e @ ctx)/sums @ WVO + bo''    WVO = (Wo @ Wv)^T [C, D]
                                          bo''= bo + Wo @ bv

FLOPs/core: 4.83 GMAC vs 16.1 GMAC for the naive pipeline.

Dtypes: bf16 operands everywhere (fp32 PSUM accumulation); the scores
matmul runs in fp8e4m3 with DoubleRow perf mode (2x PE throughput), with
xm/ctx held UNSCALED (sigma ~0.3-1, e4m3 normal range) and the 1/sqrt(D)
folded into the Exp activation. Measured end-to-end scale_rel ~1.2e-2
(tolerance 2e-2); set SCORES_FP8=False for a ~2e-3, slightly slower build.

Per-core phases:
  phase 1 (per 512-wide q chunk): xm^T[c,q] = M^T x^T + bqk, fp8 resident.
  phase 2 (per 512-wide q block):
    scores^T[k,q] accumulated over c (fp8 DoubleRow) -> exp -> expt bf16;
    DVE accumulates the softmax denominator across k-tiles; one fp32
    ones-matmul + e0-trick transpose -> 1/sums per q-partition.
    outp^T[c,q] = ctx_k^T @ expT accumulated over k.
    final[q,o] = outp^T.T @ WVO; evac fuses (*recip + bo'') in one DVE op.
"""

import numpy as np
import ml_dtypes

import concourse.bass as bass  # noqa: F401
import concourse.mybir as mybir
import concourse.tile as tile
from concourse import bacc
from concourse.bass_utils import run_bass_kernel_spmd

# ---- problem shapes (hardcoded) ----
B, SQ, SKV, D, C = 8, 2048, 2048, 1024, 768
P = 128
DT = D // P          # 8  d-tiles
CT = C // P          # 6  c-tiles
KT = SKV // P        # 16 k-tiles
QB = 512             # q block width
NQB = SQ // QB       # 4 q blocks
SCALE = 1.0 / np.sqrt(np.float32(D))

F32 = mybir.dt.float32
BF16 = mybir.dt.bfloat16
FP8 = mybir.dt.float8e4
AF = mybir.ActivationFunctionType
ALU = mybir.AluOpType
DR = mybir.MatmulPerfMode.DoubleRow

SCORES_FP8 = True

_NC_CACHE = {}


def build():
    if "nc" in _NC_CACHE:
        return _NC_CACHE["nc"]
    nc = bacc.Bacc(trn_type="TRN2", num_swdge_queues=4)

    KQDT = FP8 if SCORES_FP8 else BF16

    # ---- DRAM I/O (per-core slices; names = in_map keys) ----
    xT = nc.dram_tensor("xT", [D, SQ], BF16, kind="ExternalInput")
    ctx8T = nc.dram_tensor("ctx8T", [C, SKV], KQDT, kind="ExternalInput")
    ctxk = nc.dram_tensor("ctxk", [SKV, C], BF16, kind="ExternalInput")
    Mh = nc.dram_tensor("Mh", [D, C], BF16, kind="ExternalInput")
    wvoh = nc.dram_tensor("wvoh", [C, D], BF16, kind="ExternalInput")
    bqkh = nc.dram_tensor("bqkh", [P, CT], F32, kind="ExternalInput")
    bob = nc.dram_tensor("bob", [P, D], F32, kind="ExternalInput")
    onesmat = nc.dram_tensor("onesmat", [P, P], F32, kind="ExternalInput")
    e0two = nc.dram_tensor("e0two", [P, 2], F32, kind="ExternalInput")
    out = nc.dram_tensor("out", [SQ, D], F32, kind="ExternalOutput")

    with tile.TileContext(nc) as tc:
        with tc.tile_pool(name="persist", bufs=1) as persist:
            ctx8_sb = persist.tile([P, CT, SKV], KQDT, name="ctx8_sb")
            ctxk_sb = persist.tile([P, KT, C], BF16, name="ctxk_sb")   # 24KB/p
            xm_sb = persist.tile([P, CT, SQ], KQDT, name="xm_sb")
            m_sb = persist.tile([P, DT, C], BF16, name="m_sb")         # 12KB/p
            wvo_sb = persist.tile([P, CT, D], BF16, name="wvo_sb")     # 12KB/p
            bqk_sb = persist.tile([P, CT], F32, name="bqk_sb")
            bo_sb = persist.tile([P, D], F32, name="bo_sb")
            om_sb = persist.tile([P, P], F32, name="om_sb")
            e0_sb = persist.tile([P, 2], F32, name="e0_sb")
            sums_sb = persist.tile([P, QB], F32, name="sums_sb")

            with tc.tile_pool(name="p1_s", bufs=4) as p1_s:
                xt_tiles = [p1_s.tile([P, DT, QB], BF16, name="xt_sb",
                                      tag="xt") for qc in range(NQB)]
                # need-order DMA fanned across the three issue-capable
                # engines (sync/SP, scalar/ACT, gpsimd); first xm group's
                # operands go first
                for it in range(DT):
                    nc.sync.dma_start(m_sb[:, it], Mh[it * P:(it + 1) * P, :])
                    nc.sync.dma_start(xt_tiles[0][:, it],
                                      xT[it * P:(it + 1) * P, 0:QB])
                nc.sync.dma_start(bqk_sb, bqkh[:])
                nc.sync.dma_start(bo_sb, bob[:])
                nc.sync.dma_start(om_sb, onesmat[:])
                nc.sync.dma_start(e0_sb, e0two[:])
                for t in range(CT):
                    nc.scalar.dma_start(ctx8_sb[:, t],
                                        ctx8T[t * P:(t + 1) * P, :])
                for t in range(CT):
                    nc.scalar.dma_start(wvo_sb[:, t],
                                        wvoh[t * P:(t + 1) * P, :])
                for kt_ in range(KT):
                    nc.gpsimd.dma_start(ctxk_sb[:, kt_],
                                        ctxk[kt_ * P:(kt_ + 1) * P, :])
                for qc in range(1, NQB):
                    eng = (nc.gpsimd, nc.sync, nc.gpsimd)[qc - 1]
                    for it in range(DT):
                        eng.dma_start(xt_tiles[qc][:, it],
                                      xT[it * P:(it + 1) * P,
                                         qc * QB:(qc + 1) * QB])

                # ===== phase 1: xm^T[c,q] = M^T @ x^T (+bqk), resident =====
                with tc.tile_pool(name="ps_xm", bufs=2, space="PSUM") as ps_xm:
                    for qc in range(NQB):
                        for cs in range(CT):
                            pxm = ps_xm.tile([P, QB], F32, name="pxm", tag="pxm")
                            for it in range(DT):
                                nc.tensor.matmul(
                                    pxm, m_sb[:, it, cs * P:(cs + 1) * P],
                                    xt_tiles[qc][:, it],
                                    start=(it == 0), stop=(it == DT - 1))
                            nc.scalar.activation(
                                xm_sb[:, cs, qc * QB:(qc + 1) * QB], pxm,
                                AF.Identity, bias=bqk_sb[:, cs:cs + 1])

            # ================= phase 2: attention + fold-out ================
            with tc.tile_pool(name="p2_big", bufs=1) as p2_big, \
                 tc.tile_pool(name="p2_acc", bufs=2) as p2_acc, \
                 tc.tile_pool(name="p2_fin", bufs=4) as p2_fin, \
                 tc.tile_pool(name="p2_rcp", bufs=2) as p2_rcp, \
                 tc.tile_pool(name="ps_sc", bufs=2, space="PSUM") as ps_sc, \
                 tc.tile_pool(name="ps_sum", bufs=1, space="PSUM") as ps_sum, \
                 tc.tile_pool(name="ps_rt", bufs=1, space="PSUM") as ps_rt, \
                 tc.tile_pool(name="ps_po", bufs=2, space="PSUM") as ps_po, \
                 tc.tile_pool(name="ps_fin", bufs=2, space="PSUM") as ps_fin:
                expt_sb = p2_big.tile([P, KT, QB], BF16, name="expt_sb")
                outp_sb = p2_big.tile([P, CT, QB], BF16, name="outp_sb")
                for qb in range(NQB):
                    # ---- scores^T + exp; DVE accumulates denominator ----
                    acc = None
                    for kt_ in range(KT):
                        psc = ps_sc.tile([P, QB], F32, name="psc", tag="psc")
                        if SCORES_FP8:
                            for cs in range(0, CT, 2):
                                nc.tensor.matmul(
                                    psc,
                                    ctx8_sb[:, cs:cs + 2, kt_ * P:(kt_ + 1) * P],
                                    xm_sb[:, cs:cs + 2, qb * QB:(qb + 1) * QB],
                                    start=(cs == 0), stop=(cs == CT - 2),
                                    perf_mode=DR)
                        else:
                            for cs in range(CT):
                                nc.tensor.matmul(
                                    psc, ctx8_sb[:, cs, kt_ * P:(kt_ + 1) * P],
                                    xm_sb[:, cs, qb * QB:(qb + 1) * QB],
                                    start=(cs == 0), stop=(cs == CT - 1))
                        nc.scalar.activation(
                            expt_sb[:, kt_], psc, AF.Exp,
                            scale=float(SCALE) if SCORES_FP8 else 1.0)
                        nacc = p2_acc.tile([P, QB], F32, name="acc", tag="acc")
                        if kt_ == 0:
                            nc.vector.tensor_scalar_mul(nacc, expt_sb[:, 0], 1.0)
                        else:
                            nc.vector.tensor_add(nacc, acc, expt_sb[:, kt_])
                        acc = nacc
                    # ---- outp^T[c,q] = ctx_k^T @ expT, in cs pairs ----
                    def outp_pair(cp):
                        po0 = ps_po.tile([P, QB], F32, name="po0", tag="po")
                        po1 = ps_po.tile([P, QB], F32, name="po1", tag="po")
                        po = (po0, po1)
                        for kt_ in range(KT):
                            for cc in range(2):
                                c0 = (cp * 2 + cc) * P
                                nc.tensor.matmul(
                                    po[cc], ctxk_sb[:, kt_, c0:c0 + P],
                                    expt_sb[:, kt_],
                                    start=(kt_ == 0), stop=(kt_ == KT - 1))
                        for cc in range(2):
                            nc.scalar.copy(outp_sb[:, cp * 2 + cc], po[cc])
                    outp_pair(0)
                    # ---- sums: fp32 partition-reduce + e0-trick transpose ---
                    psums = ps_sum.tile([P, QB], F32, name="psums", tag="psums")
                    nc.tensor.matmul(psums, om_sb, acc, start=True, stop=True)
                    nc.scalar.copy(sums_sb, psums)
                    prt = ps_rt.tile([P, 8], F32, name="prt", tag="prt")
                    for qs in range(4):
                        nc.tensor.matmul(
                            prt[:, 2 * qs:2 * qs + 2],
                            sums_sb[:, qs * P:(qs + 1) * P], e0_sb,
                            start=True, stop=True)
                    recip = p2_rcp.tile([P, 8], F32, name="recip", tag="recip")
                    nc.vector.reciprocal(recip, prt)
                    for cp in range(1, 3):
                        outp_pair(cp)
                    # ---- final = outp^T.T @ WVO; evac fuses *recip + bo'' ---
                    for qs in range(4):
                        for oc in range(2):
                            pf = ps_fin.tile([P, 512], F32, name="pf", tag="pf")
                            for cs in range(CT):
                                nc.tensor.matmul(
                                    pf, outp_sb[:, cs, qs * P:(qs + 1) * P],
                                    wvo_sb[:, cs, oc * 512:(oc + 1) * 512],
                                    start=(cs == 0), stop=(cs == CT - 1))
                            fin = p2_fin.tile([P, 512], F32, name="fin",
                                              tag="fin")
                            nc.vector.scalar_tensor_tensor(
                                fin, pf, recip[:, 2 * qs:2 * qs + 1],
                                bo_sb[:, oc * 512:(oc + 1) * 512],
                                op0=ALU.mult, op1=ALU.add)
                            seng = nc.gpsimd if (qs * 2 + oc) % 2 else nc.sync
                            seng.dma_start(
                                out[qb * QB + qs * P: qb * QB + (qs + 1) * P,
                                    oc * 512:(oc + 1) * 512], fin)
    nc.finalize()
    _NC_CACHE["nc"] = nc
    return nc


def _host_prep(x, context, Wq, bq, Wk, bk, Wv, bv, Wo, bo):
    """Build the 8 per-core input maps (host-side weight folding)."""
    BF = ml_dtypes.bfloat16
    F8np = ml_dtypes.float8_e4m3
    x = np.asarray(x, dtype=np.float32)
    context = np.asarray(context, dtype=np.float32)
    Wq64 = np.asarray(Wq, np.float64)
    Wk64 = np.asarray(Wk, np.float64)
    Wv64 = np.asarray(Wv, np.float64)
    Wo64 = np.asarray(Wo, np.float64)
    scale = np.float64(1.0) if SCORES_FP8 else np.float64(SCALE)
    M = (Wq64.T @ Wk64) * scale                       # [D, C]
    bqk = (np.asarray(bq, np.float64) @ Wk64) * scale  # [C]
    WVO = (Wo64 @ Wv64).T                             # [C, D]
    bo_eff = np.asarray(bo, np.float64) + Wo64 @ np.asarray(bv, np.float64)

    Mh = np.ascontiguousarray(M.astype(np.float32)).astype(BF)
    wvoh = np.ascontiguousarray(WVO.astype(np.float32)).astype(BF)
    bqkh = np.ascontiguousarray(
        bqk.astype(np.float32).reshape(CT, P).T)      # [p, ct]
    bob = np.ascontiguousarray(
        np.broadcast_to(bo_eff.astype(np.float32)[None, :], (P, D)))
    onesmat = np.ones((P, P), np.float32)
    e0two = np.zeros((P, 2), np.float32)
    e0two[0, :] = 1.0
    shared = dict(Mh=Mh, wvoh=wvoh, bqkh=bqkh, bob=bob,
                  onesmat=onesmat, e0two=e0two)
    xbf = x.astype(BF)
    cbf = context.astype(BF)
    in_maps = []
    for b in range(B):
        m = dict(shared)
        m["xT"] = np.ascontiguousarray(xbf[b].T)              # [D, SQ] bf16
        ctxTb = np.ascontiguousarray(cbf[b].T)                # [C, SKV]
        m["ctx8T"] = ctxTb.astype(F8np) if SCORES_FP8 else ctxTb
        m["ctxk"] = np.ascontiguousarray(cbf[b])              # [SKV, C] bf16
        in_maps.append(m)
    return in_maps


def kernel(**inputs) -> np.ndarray:
    nc = build()
    in_maps = _host_prep(**inputs)
    res = run_bass_kernel_spmd(nc, in_maps, core_ids=list(range(B)))
    return np.stack([res.results[b]["out"] for b in range(B)], axis=0)


# revision 19
# speedup vs baseline: 2.1024x; 1.0055x over previous
"""Trainium2 Bass kernel for nn_CrossAttention (B=8, Sq=Skv=2048, D=1024, C=768).

Strategy: data-parallel over batch — each of the 8 NeuronCores computes one
batch element's full cross-attention.

The projection chain is reassociated so every big contraction runs against
the NARROW context dim (C=768) instead of D=1024, and the K/V projections
disappear entirely (all exact identities, weights folded on host):

  scores = (x @ M + bqk) @ ctx^T          M   = Wq^T @ Wk   [D, C]
                                          bqk = bq @ Wk     [C]
  (bk drops: its score term is constant over k -> cancels in softmax)
  att    = softmax(scores / sqrt(D))
  final  = (e @ ctx)/sums @ WVO + bo''    WVO = (Wo @ Wv)^T [C, D]
                                          bo''= bo + Wo @ bv

FLOPs/core: 4.83 GMAC vs 16.1 GMAC for the naive pipeline.

Dtypes: bf16 operands everywhere (fp32 PSUM accumulation); the scores
matmul runs in fp8e4m3 with DoubleRow perf mode (2x PE throughput), with
xm/ctx held UNSCALED (sigma ~0.3-1, e4m3 normal range) and the 1/sqrt(D)
folded into the Exp activation. Measured end-to-end scale_rel ~1.2e-2
(tolerance 2e-2); set SCORES_FP8=False for a ~2e-3, slightly slower build.

Per-core phases:
  phase 1 (per 512-wide q chunk): xm^T[c,q] = M^T x^T + bqk, fp8 resident.
  phase 2 (per 512-wide q block):
    scores^T[k,q] accumulated over c (fp8 DoubleRow) -> exp -> expt bf16;
    DVE accumulates the softmax denominator across k-tiles; one fp32
    ones-matmul + e0-trick transpose -> 1/sums per q-partition.
    outp^T[c,q] = ctx_k^T @ expT accumulated over k.
    final[q,o] = outp^T.T @ WVO; evac fuses (*recip + bo'') in one DVE op.
"""

import numpy as np
import ml_dtypes

import concourse.bass as bass  # noqa: F401
import concourse.mybir as mybir
import concourse.tile as tile
from concourse import bacc
from concourse.bass_utils import run_bass_kernel_spmd

# ---- problem shapes (hardcoded) ----
B, SQ, SKV, D, C = 8, 2048, 2048, 1024, 768
P = 128
DT = D // P          # 8  d-tiles
CT = C // P          # 6  c-tiles
KT = SKV // P        # 16 k-tiles
QB = 512             # q block width
NQB = SQ // QB       # 4 q blocks
SCALE = 1.0 / np.sqrt(np.float32(D))

F32 = mybir.dt.float32
BF16 = mybir.dt.bfloat16
FP8 = mybir.dt.float8e4
AF = mybir.ActivationFunctionType
ALU = mybir.AluOpType
DR = mybir.MatmulPerfMode.DoubleRow

SCORES_FP8 = True

_NC_CACHE = {}


def build():
    if "nc" in _NC_CACHE:
        return _NC_CACHE["nc"]
    nc = bacc.Bacc(trn_type="TRN2", num_swdge_queues=4)

    KQDT = FP8 if SCORES_FP8 else BF16

    # ---- DRAM I/O (per-core slices; names = in_map keys) ----
    xT = nc.dram_tensor("xT", [D, SQ], BF16, kind="ExternalInput")
    ctx8T = nc.dram_tensor("ctx8T", [C, SKV], KQDT, kind="ExternalInput")
    ctxk = nc.dram_tensor("ctxk", [SKV, C], BF16, kind="ExternalInput")
    Mh = nc.dram_tensor("Mh", [D, C], BF16, kind="ExternalInput")
    wvoh = nc.dram_tensor("wvoh", [C, D], BF16, kind="ExternalInput")
    bqkh = nc.dram_tensor("bqkh", [P, CT], F32, kind="ExternalInput")
    bob = nc.dram_tensor("bob", [P, D], F32, kind="ExternalInput")
    onesmat = nc.dram_tensor("onesmat", [P, P], F32, kind="ExternalInput")
    e0two = nc.dram_tensor("e0two", [P, 2], F32, kind="ExternalInput")
    out = nc.dram_tensor("out", [SQ, D], F32, kind="ExternalOutput")

    with tile.TileContext(nc) as tc:
        with tc.tile_pool(name="persist", bufs=1) as persist:
            ctx8_sb = persist.tile([P, CT, SKV], KQDT, name="ctx8_sb")
            ctxk_sb = persist.tile([P, KT, C], BF16, name="ctxk_sb")   # 24KB/p
            xm_sb = persist.tile([P, CT, SQ], KQDT, name="xm_sb")
            m_sb = persist.tile([P, DT, C], BF16, name="m_sb")         # 12KB/p
            wvo_sb = persist.tile([P, CT, D], BF16, name="wvo_sb")     # 12KB/p
            bqk_sb = persist.tile([P, CT], F32, name="bqk_sb")
            bo_sb = persist.tile([P, D], F32, name="bo_sb")
            om_sb = persist.tile([P, P], F32, name="om_sb")
            e0_sb = persist.tile([P, 2], F32, name="e0_sb")
            sums_sb = persist.tile([P, QB], F32, name="sums_sb")

            with tc.tile_pool(name="p1_s", bufs=4) as p1_s:
                xt_tiles = [p1_s.tile([P, DT, QB], BF16, name="xt_sb",
                                      tag="xt") for qc in range(NQB)]
                # need-order DMA fanned across the three issue-capable
                # engines (sync/SP, scalar/ACT, gpsimd); first xm group's
                # operands go first
                # x and M gate phase 1 — they get all three engines' queue
                # groups first; ctx8/ctxk/wvo are needed only at ~50/65/80us
                for it in range(DT):
                    nc.sync.dma_start(m_sb[:, it], Mh[it * P:(it + 1) * P, :])
                    nc.sync.dma_start(xt_tiles[0][:, it],
                                      xT[it * P:(it + 1) * P, 0:QB])
                    nc.scalar.dma_start(xt_tiles[1][:, it],
                                        xT[it * P:(it + 1) * P, QB:2 * QB])
                    nc.gpsimd.dma_start(xt_tiles[2][:, it],
                                        xT[it * P:(it + 1) * P, 2 * QB:3 * QB])
                nc.sync.dma_start(bqk_sb, bqkh[:])
                nc.sync.dma_start(bo_sb, bob[:])
                nc.sync.dma_start(om_sb, onesmat[:])
                nc.sync.dma_start(e0_sb, e0two[:])
                for it in range(DT):
                    nc.sync.dma_start(xt_tiles[3][:, it],
                                      xT[it * P:(it + 1) * P, 3 * QB:4 * QB])
                for t in range(CT):
                    nc.scalar.dma_start(ctx8_sb[:, t],
                                        ctx8T[t * P:(t + 1) * P, :])
                for kt_ in range(KT):
                    nc.gpsimd.dma_start(ctxk_sb[:, kt_],
                                        ctxk[kt_ * P:(kt_ + 1) * P, :])
                for t in range(CT):
                    nc.scalar.dma_start(wvo_sb[:, t],
                                        wvoh[t * P:(t + 1) * P, :])

                # ===== phase 1: xm^T[c,q] = M^T @ x^T (+bqk), resident =====
                with tc.tile_pool(name="ps_xm", bufs=2, space="PSUM") as ps_xm:
                    for qc in range(NQB):
                        for cs in range(CT):
                            pxm = ps_xm.tile([P, QB], F32, name="pxm", tag="pxm")
                            for it in range(DT):
                                nc.tensor.matmul(
                                    pxm, m_sb[:, it, cs * P:(cs + 1) * P],
                                    xt_tiles[qc][:, it],
                                    start=(it == 0), stop=(it == DT - 1))
                            nc.scalar.activation(
                                xm_sb[:, cs, qc * QB:(qc + 1) * QB], pxm,
                                AF.Identity, bias=bqk_sb[:, cs:cs + 1])

            # ================= phase 2: attention + fold-out ================
            with tc.tile_pool(name="p2_big", bufs=1) as p2_big, \
                 tc.tile_pool(name="p2_acc", bufs=2) as p2_acc, \
                 tc.tile_pool(name="p2_fin", bufs=4) as p2_fin, \
                 tc.tile_pool(name="p2_rcp", bufs=2) as p2_rcp, \
                 tc.tile_pool(name="ps_sc", bufs=3, space="PSUM") as ps_sc, \
                 tc.tile_pool(name="ps_sum", bufs=1, space="PSUM") as ps_sum, \
                 tc.tile_pool(name="ps_po", bufs=2, space="PSUM") as ps_po, \
                 tc.tile_pool(name="ps_fin", bufs=2, space="PSUM") as ps_fin:
                expt_sb = p2_big.tile([P, KT, QB], BF16, name="expt_sb")
                outp_sb = p2_big.tile([P, CT, QB], BF16, name="outp_sb")
                for qb in range(NQB):
                    # ---- scores^T + exp; DVE accumulates denominator ----
                    acc = None
                    for kt_ in range(KT):
                        psc = ps_sc.tile([P, QB], F32, name="psc", tag="psc")
                        if SCORES_FP8:
                            for cs in range(0, CT, 2):
                                nc.tensor.matmul(
                                    psc,
                                    ctx8_sb[:, cs:cs + 2, kt_ * P:(kt_ + 1) * P],
                                    xm_sb[:, cs:cs + 2, qb * QB:(qb + 1) * QB],
                                    start=(cs == 0), stop=(cs == CT - 2),
                                    perf_mode=DR)
                        else:
                            for cs in range(CT):
                                nc.tensor.matmul(
                                    psc, ctx8_sb[:, cs, kt_ * P:(kt_ + 1) * P],
                                    xm_sb[:, cs, qb * QB:(qb + 1) * QB],
                                    start=(cs == 0), stop=(cs == CT - 1))
                        nc.scalar.activation(
                            expt_sb[:, kt_], psc, AF.Exp,
                            scale=float(SCALE) if SCORES_FP8 else 1.0)
                        nacc = p2_acc.tile([P, QB], F32, name="acc", tag="acc")
                        if kt_ == 0:
                            nc.vector.tensor_scalar_mul(nacc, expt_sb[:, 0], 1.0)
                        else:
                            nc.vector.tensor_add(nacc, acc, expt_sb[:, kt_])
                        acc = nacc
                    # ---- outp^T[c,q] = ctx_k^T @ expT, in cs pairs ----
                    def outp_pair(cp):
                        po0 = ps_po.tile([P, QB], F32, name="po0", tag="po")
                        po1 = ps_po.tile([P, QB], F32, name="po1", tag="po")
                        po = (po0, po1)
                        for kt_ in range(KT):
                            for cc in range(2):
                                c0 = (cp * 2 + cc) * P
                                nc.tensor.matmul(
                                    po[cc], ctxk_sb[:, kt_, c0:c0 + P],
                                    expt_sb[:, kt_],
                                    start=(kt_ == 0), stop=(kt_ == KT - 1))
                        for cc in range(2):
                            nc.scalar.copy(outp_sb[:, cp * 2 + cc], po[cc])
                    outp_pair(0)
                    # ---- sums: fp32 partition-reduce + e0-trick transpose ---
                    # psums and prt share one PSUM pool slot: psums is dead
                    # once copied to sums_sb, freeing the bank for prt
                    psums = ps_sum.tile([P, QB], F32, name="psums", tag="ps_r")
                    nc.tensor.matmul(psums, om_sb, acc, start=True, stop=True)
                    nc.scalar.copy(sums_sb, psums)
                    prt = ps_sum.tile([P, 8], F32, name="prt", tag="ps_r")
                    for qs in range(4):
                        nc.tensor.matmul(
                            prt[:, 2 * qs:2 * qs + 2],
                            sums_sb[:, qs * P:(qs + 1) * P], e0_sb,
                            start=True, stop=True)
                    recip = p2_rcp.tile([P, 8], F32, name="recip", tag="recip")
                    nc.vector.reciprocal(recip, prt)
                    for cp in range(1, 3):
                        outp_pair(cp)
                    # ---- final = outp^T.T @ WVO; evac fuses *recip + bo'' ---
                    for qs in range(4):
                        for oc in range(2):
                            pf = ps_fin.tile([P, 512], F32, name="pf", tag="pf")
                            for cs in range(CT):
                                nc.tensor.matmul(
                                    pf, outp_sb[:, cs, qs * P:(qs + 1) * P],
                                    wvo_sb[:, cs, oc * 512:(oc + 1) * 512],
                                    start=(cs == 0), stop=(cs == CT - 1))
                            fin = p2_fin.tile([P, 512], F32, name="fin",
                                              tag="fin")
                            nc.vector.scalar_tensor_tensor(
                                fin, pf, recip[:, 2 * qs:2 * qs + 1],
                                bo_sb[:, oc * 512:(oc + 1) * 512],
                                op0=ALU.mult, op1=ALU.add)
                            seng = nc.gpsimd if (qs * 2 + oc) % 2 else nc.sync
                            seng.dma_start(
                                out[qb * QB + qs * P: qb * QB + (qs + 1) * P,
                                    oc * 512:(oc + 1) * 512], fin)
    nc.finalize()
    _NC_CACHE["nc"] = nc
    return nc


def _host_prep(x, context, Wq, bq, Wk, bk, Wv, bv, Wo, bo):
    """Build the 8 per-core input maps (host-side weight folding)."""
    BF = ml_dtypes.bfloat16
    F8np = ml_dtypes.float8_e4m3
    x = np.asarray(x, dtype=np.float32)
    context = np.asarray(context, dtype=np.float32)
    Wq64 = np.asarray(Wq, np.float64)
    Wk64 = np.asarray(Wk, np.float64)
    Wv64 = np.asarray(Wv, np.float64)
    Wo64 = np.asarray(Wo, np.float64)
    scale = np.float64(1.0) if SCORES_FP8 else np.float64(SCALE)
    M = (Wq64.T @ Wk64) * scale                       # [D, C]
    bqk = (np.asarray(bq, np.float64) @ Wk64) * scale  # [C]
    WVO = (Wo64 @ Wv64).T                             # [C, D]
    bo_eff = np.asarray(bo, np.float64) + Wo64 @ np.asarray(bv, np.float64)

    Mh = np.ascontiguousarray(M.astype(np.float32)).astype(BF)
    wvoh = np.ascontiguousarray(WVO.astype(np.float32)).astype(BF)
    bqkh = np.ascontiguousarray(
        bqk.astype(np.float32).reshape(CT, P).T)      # [p, ct]
    bob = np.ascontiguousarray(
        np.broadcast_to(bo_eff.astype(np.float32)[None, :], (P, D)))
    onesmat = np.ones((P, P), np.float32)
    e0two = np.zeros((P, 2), np.float32)
    e0two[0, :] = 1.0
    shared = dict(Mh=Mh, wvoh=wvoh, bqkh=bqkh, bob=bob,
                  onesmat=onesmat, e0two=e0two)
    xbf = x.astype(BF)
    cbf = context.astype(BF)
    in_maps = []
    for b in range(B):
        m = dict(shared)
        m["xT"] = np.ascontiguousarray(xbf[b].T)              # [D, SQ] bf16
        ctxTb = np.ascontiguousarray(cbf[b].T)                # [C, SKV]
        m["ctx8T"] = ctxTb.astype(F8np) if SCORES_FP8 else ctxTb
        m["ctxk"] = np.ascontiguousarray(cbf[b])              # [SKV, C] bf16
        in_maps.append(m)
    return in_maps


def kernel(**inputs) -> np.ndarray:
    nc = build()
    in_maps = _host_prep(**inputs)
    res = run_bass_kernel_spmd(nc, in_maps, core_ids=list(range(B)))
    return np.stack([res.results[b]["out"] for b in range(B)], axis=0)


# revision 27
# speedup vs baseline: 2.2197x; 1.0558x over previous
"""Trainium2 Bass kernel for nn_CrossAttention (B=8, Sq=Skv=2048, D=1024, C=768).

Strategy: data-parallel over batch — each of the 8 NeuronCores computes one
batch element's full cross-attention.

The projection chain is reassociated so every big contraction runs against
the NARROW context dim (C=768) instead of D=1024, and the K/V projections
disappear entirely (all exact identities, weights folded on host):

  scores = (x @ M + bqk) @ ctx^T          M   = Wq^T @ Wk   [D, C]
                                          bqk = bq @ Wk     [C]
  (bk drops: its score term is constant over k -> cancels in softmax)
  att    = softmax(scores / sqrt(D))
  final  = (e @ ctx)/sums @ WVO + bo''    WVO = (Wo @ Wv)^T [C, D]
                                          bo''= bo + Wo @ bv

FLOPs/core: 4.83 GMAC vs 16.1 GMAC for the naive pipeline.

Dtypes: bf16 operands everywhere (fp32 PSUM accumulation); the scores
matmul runs in fp8e4m3 with DoubleRow perf mode (2x PE throughput), with
xm/ctx held UNSCALED (sigma ~0.3-1, e4m3 normal range) and the 1/sqrt(D)
folded into the Exp activation. Measured end-to-end scale_rel ~1.2e-2
(tolerance 2e-2); set SCORES_FP8=False for a ~2e-3, slightly slower build.

Per-core phases:
  phase 1 (per 512-wide q chunk): xm^T[c,q] = M^T x^T + bqk, fp8 resident.
  phase 2 (per 512-wide q block):
    scores^T[k,q] accumulated over c (fp8 DoubleRow) -> exp -> expt bf16;
    DVE accumulates the softmax denominator across k-tiles; one fp32
    ones-matmul + e0-trick transpose -> 1/sums per q-partition.
    outp^T[c,q] = ctx_k^T @ expT accumulated over k.
    final[q,o] = outp^T.T @ WVO; evac fuses (*recip + bo'') in one DVE op.
"""

import numpy as np
import ml_dtypes

import concourse.bass as bass  # noqa: F401
import concourse.mybir as mybir
import concourse.tile as tile
from concourse import bacc
from concourse.bass_utils import run_bass_kernel_spmd

# ---- problem shapes (hardcoded) ----
B, SQ, SKV, D, C = 8, 2048, 2048, 1024, 768
P = 128
DT = D // P          # 8  d-tiles
CT = C // P          # 6  c-tiles
KT = SKV // P        # 16 k-tiles
QB = 512             # q block width
NQB = SQ // QB       # 4 q blocks
SCALE = 1.0 / np.sqrt(np.float32(D))

F32 = mybir.dt.float32
BF16 = mybir.dt.bfloat16
FP8 = mybir.dt.float8e4
AF = mybir.ActivationFunctionType
ALU = mybir.AluOpType
DR = mybir.MatmulPerfMode.DoubleRow

SCORES_FP8 = True

_NC_CACHE = {}


def build():
    if "nc" in _NC_CACHE:
        return _NC_CACHE["nc"]
    nc = bacc.Bacc(trn_type="TRN2", num_swdge_queues=4)

    KQDT = FP8 if SCORES_FP8 else BF16

    # ---- DRAM I/O (per-core slices; names = in_map keys) ----
    xT = nc.dram_tensor("xT", [D, SQ], BF16, kind="ExternalInput")
    ctx8T = nc.dram_tensor("ctx8T", [C, SKV], KQDT, kind="ExternalInput")
    ctxk = nc.dram_tensor("ctxk", [SKV, C], BF16, kind="ExternalInput")
    Mh = nc.dram_tensor("Mh", [D, C], BF16, kind="ExternalInput")
    wvoh = nc.dram_tensor("wvoh", [C, D], BF16, kind="ExternalInput")
    bqkh = nc.dram_tensor("bqkh", [P, CT], F32, kind="ExternalInput")
    bob = nc.dram_tensor("bob", [P, D], F32, kind="ExternalInput")
    onesmat = nc.dram_tensor("onesmat", [P, P], F32, kind="ExternalInput")
    e0two = nc.dram_tensor("e0two", [P, 2], F32, kind="ExternalInput")
    out = nc.dram_tensor("out", [SQ, D], F32, kind="ExternalOutput")

    with tile.TileContext(nc) as tc:
        with tc.tile_pool(name="persist", bufs=1) as persist:
            ctx8_sb = persist.tile([P, CT, SKV], KQDT, name="ctx8_sb")
            ctxk_sb = persist.tile([P, KT, C], BF16, name="ctxk_sb")   # 24KB/p
            xm_sb = persist.tile([P, CT, SQ], KQDT, name="xm_sb")
            m_sb = persist.tile([P, DT, C], BF16, name="m_sb")         # 12KB/p
            wvo_sb = persist.tile([P, CT, D], BF16, name="wvo_sb")     # 12KB/p
            bqk_sb = persist.tile([P, CT], F32, name="bqk_sb")
            bo_sb = persist.tile([P, D], F32, name="bo_sb")
            om_sb = persist.tile([P, P], F32, name="om_sb")
            e0_sb = persist.tile([P, 2], F32, name="e0_sb")
            sums_sb = persist.tile([P, QB], F32, name="sums_sb")

            with tc.tile_pool(name="p1_s", bufs=4) as p1_s:
                xt_tiles = [p1_s.tile([P, DT, QB], BF16, name="xt_sb",
                                      tag="xt") for qc in range(NQB)]
                # need-order DMA fanned across the three issue-capable
                # engines (sync/SP, scalar/ACT, gpsimd); first xm group's
                # operands go first
                # x and M gate phase 1 — they get all three engines' queue
                # groups first; ctx8/ctxk/wvo are needed only at ~50/65/80us
                for it in range(DT):
                    nc.sync.dma_start(m_sb[:, it], Mh[it * P:(it + 1) * P, :])
                    nc.sync.dma_start(xt_tiles[0][:, it],
                                      xT[it * P:(it + 1) * P, 0:QB])
                    nc.scalar.dma_start(xt_tiles[1][:, it],
                                        xT[it * P:(it + 1) * P, QB:2 * QB])
                    nc.gpsimd.dma_start(xt_tiles[2][:, it],
                                        xT[it * P:(it + 1) * P, 2 * QB:3 * QB])
                nc.sync.dma_start(bqk_sb, bqkh[:])
                nc.sync.dma_start(bo_sb, bob[:])
                nc.sync.dma_start(om_sb, onesmat[:])
                nc.sync.dma_start(e0_sb, e0two[:])
                for it in range(DT):
                    nc.gpsimd.dma_start(xt_tiles[3][:, it],
                                        xT[it * P:(it + 1) * P, 3 * QB:4 * QB])

                # ===== phase 1: xm^T[c,q] = M^T @ x^T (+bqk), resident =====
                # ctx8/ctxk/wvo (6MB, needed at ~50/62/70us) are issued from
                # the scalar engine's stream AFTER per-chunk compute
                # milestones, so their transfers don't steal HBM bandwidth
                # from the x/M stream that gates this phase.
                with tc.tile_pool(name="ps_xm", bufs=3, space="PSUM") as ps_xm:
                    for qc in range(NQB):
                        for cs in range(CT):
                            pxm = ps_xm.tile([P, QB], F32, name="pxm", tag="pxm")
                            for it in range(DT):
                                nc.tensor.matmul(
                                    pxm, m_sb[:, it, cs * P:(cs + 1) * P],
                                    xt_tiles[qc][:, it],
                                    start=(it == 0), stop=(it == DT - 1))
                            nc.scalar.activation(
                                xm_sb[:, cs, qc * QB:(qc + 1) * QB], pxm,
                                AF.Identity, bias=bqk_sb[:, cs:cs + 1])
                        # ~5 issues per chunk keeps the scalar engine's evac
                        # cadence ahead of the PE
                        if qc == 0:
                            for t in range(CT):
                                nc.scalar.dma_start(ctx8_sb[:, t],
                                                    ctx8T[t * P:(t + 1) * P, :])
                        elif qc in (1, 2):
                            for kt_ in range((qc - 1) * 8, (qc - 1) * 8 + 8):
                                nc.scalar.dma_start(
                                    ctxk_sb[:, kt_],
                                    ctxk[kt_ * P:(kt_ + 1) * P, :])
                        else:
                            for t in range(CT):
                                nc.scalar.dma_start(wvo_sb[:, t],
                                                    wvoh[t * P:(t + 1) * P, :])

            # ================= phase 2: attention + fold-out ================
            with tc.tile_pool(name="p2_big", bufs=1) as p2_big, \
                 tc.tile_pool(name="p2_acc", bufs=3) as p2_acc, \
                 tc.tile_pool(name="p2_fin", bufs=4) as p2_fin, \
                 tc.tile_pool(name="p2_rcp", bufs=2) as p2_rcp, \
                 tc.tile_pool(name="ps_sc", bufs=2, space="PSUM") as ps_sc, \
                 tc.tile_pool(name="ps_po", bufs=2, space="PSUM") as ps_po, \
                 tc.tile_pool(name="ps_fin", bufs=2, space="PSUM") as ps_fin:
                expt_sb = p2_big.tile([P, KT, QB], BF16, name="expt_sb")
                outp_sb = p2_big.tile([P, CT, QB], BF16, name="outp_sb")
                for qb in range(NQB):
                    # ---- scores^T + exp; DVE accumulates denominator ----
                    # kt-tiles are processed in pairs sharing a 2-bank PSUM
                    # tile so ONE Exp covers 1024 columns: the ACT chain
                    # (~880ns/issue) stops pacing the PE's 650ns/group rate
                    acc = None
                    for kp in range(KT // 2):
                        psc = ps_sc.tile([P, 2, QB], F32, name="psc", tag="psc")
                        for j in range(2):
                            kt_ = kp * 2 + j
                            if SCORES_FP8:
                                for cs in range(0, CT, 2):
                                    nc.tensor.matmul(
                                        psc[:, j],
                                        ctx8_sb[:, cs:cs + 2, kt_ * P:(kt_ + 1) * P],
                                        xm_sb[:, cs:cs + 2, qb * QB:(qb + 1) * QB],
                                        start=(cs == 0), stop=(cs == CT - 2),
                                        perf_mode=DR)
                            else:
                                for cs in range(CT):
                                    nc.tensor.matmul(
                                        psc[:, j],
                                        ctx8_sb[:, cs, kt_ * P:(kt_ + 1) * P],
                                        xm_sb[:, cs, qb * QB:(qb + 1) * QB],
                                        start=(cs == 0), stop=(cs == CT - 1))
                        nc.scalar.activation(
                            expt_sb[:, 2 * kp:2 * kp + 2], psc, AF.Exp,
                            scale=float(SCALE) if SCORES_FP8 else 1.0)
                        pair = p2_acc.tile([P, QB], F32, name="pair", tag="acc")
                        nc.vector.tensor_add(pair, expt_sb[:, 2 * kp],
                                             expt_sb[:, 2 * kp + 1])
                        if kp == 0:
                            acc = pair
                        else:
                            nacc = p2_acc.tile([P, QB], F32, name="acc",
                                               tag="acc")
                            nc.vector.tensor_add(nacc, acc, pair)
                            acc = nacc
                    # ---- outp^T[c,q] = ctx_k^T @ expT, in cs pairs ----
                    def outp_pair(cp):
                        po0 = ps_po.tile([P, QB], F32, name="po0", tag="po")
                        po1 = ps_po.tile([P, QB], F32, name="po1", tag="po")
                        po = (po0, po1)
                        for kt_ in range(KT):
                            for cc in range(2):
                                c0 = (cp * 2 + cc) * P
                                nc.tensor.matmul(
                                    po[cc], ctxk_sb[:, kt_, c0:c0 + P],
                                    expt_sb[:, kt_],
                                    start=(kt_ == 0), stop=(kt_ == KT - 1))
                        for cc in range(2):
                            nc.scalar.copy(outp_sb[:, cp * 2 + cc], po[cc])
                    outp_pair(0)
                    # ---- sums: fp32 partition-reduce + e0-trick transpose ---
                    # psums/prt borrow the ps_fin pool: the final-projection
                    # groups only start ~15us later, so there's no overlap
                    psums = ps_fin.tile([P, QB], F32, name="psums", tag="pf")
                    nc.tensor.matmul(psums, om_sb, acc, start=True, stop=True)
                    nc.scalar.copy(sums_sb, psums)
                    prt = ps_fin.tile([P, 8], F32, name="prt", tag="pf")
                    for qs in range(4):
                        nc.tensor.matmul(
                            prt[:, 2 * qs:2 * qs + 2],
                            sums_sb[:, qs * P:(qs + 1) * P], e0_sb,
                            start=True, stop=True)
                    recip = p2_rcp.tile([P, 8], F32, name="recip", tag="recip")
                    nc.vector.reciprocal(recip, prt)
                    for cp in range(1, 3):
                        outp_pair(cp)
                    # ---- final = outp^T.T @ WVO; evac fuses *recip + bo'' ---
                    for qs in range(4):
                        for oc in range(2):
                            pf = ps_fin.tile([P, 512], F32, name="pf", tag="pf")
                            for cs in range(CT):
                                nc.tensor.matmul(
                                    pf, outp_sb[:, cs, qs * P:(qs + 1) * P],
                                    wvo_sb[:, cs, oc * 512:(oc + 1) * 512],
                                    start=(cs == 0), stop=(cs == CT - 1))
                            fin = p2_fin.tile([P, 512], F32, name="fin",
                                              tag="fin")
                            nc.vector.scalar_tensor_tensor(
                                fin, pf, recip[:, 2 * qs:2 * qs + 1],
                                bo_sb[:, oc * 512:(oc + 1) * 512],
                                op0=ALU.mult, op1=ALU.add)
                            seng = (nc.sync, nc.gpsimd, nc.scalar)[
                                (qs * 2 + oc) % 3]
                            seng.dma_start(
                                out[qb * QB + qs * P: qb * QB + (qs + 1) * P,
                                    oc * 512:(oc + 1) * 512], fin)
    nc.finalize()
    _NC_CACHE["nc"] = nc
    return nc


def _host_prep(x, context, Wq, bq, Wk, bk, Wv, bv, Wo, bo):
    """Build the 8 per-core input maps (host-side weight folding)."""
    BF = ml_dtypes.bfloat16
    F8np = ml_dtypes.float8_e4m3
    x = np.asarray(x, dtype=np.float32)
    context = np.asarray(context, dtype=np.float32)
    Wq64 = np.asarray(Wq, np.float64)
    Wk64 = np.asarray(Wk, np.float64)
    Wv64 = np.asarray(Wv, np.float64)
    Wo64 = np.asarray(Wo, np.float64)
    scale = np.float64(1.0) if SCORES_FP8 else np.float64(SCALE)
    M = (Wq64.T @ Wk64) * scale                       # [D, C]
    bqk = (np.asarray(bq, np.float64) @ Wk64) * scale  # [C]
    WVO = (Wo64 @ Wv64).T                             # [C, D]
    bo_eff = np.asarray(bo, np.float64) + Wo64 @ np.asarray(bv, np.float64)

    Mh = np.ascontiguousarray(M.astype(np.float32)).astype(BF)
    wvoh = np.ascontiguousarray(WVO.astype(np.float32)).astype(BF)
    bqkh = np.ascontiguousarray(
        bqk.astype(np.float32).reshape(CT, P).T)      # [p, ct]
    bob = np.ascontiguousarray(
        np.broadcast_to(bo_eff.astype(np.float32)[None, :], (P, D)))
    onesmat = np.ones((P, P), np.float32)
    e0two = np.zeros((P, 2), np.float32)
    e0two[0, :] = 1.0
    shared = dict(Mh=Mh, wvoh=wvoh, bqkh=bqkh, bob=bob,
                  onesmat=onesmat, e0two=e0two)
    xbf = x.astype(BF)
    cbf = context.astype(BF)
    in_maps = []
    for b in range(B):
        m = dict(shared)
        m["xT"] = np.ascontiguousarray(xbf[b].T)              # [D, SQ] bf16
        ctxTb = np.ascontiguousarray(cbf[b].T)                # [C, SKV]
        m["ctx8T"] = ctxTb.astype(F8np) if SCORES_FP8 else ctxTb
        m["ctxk"] = np.ascontiguousarray(cbf[b])              # [SKV, C] bf16
        in_maps.append(m)
    return in_maps


def kernel(**inputs) -> np.ndarray:
    nc = build()
    in_maps = _host_prep(**inputs)
    res = run_bass_kernel_spmd(nc, in_maps, core_ids=list(range(B)))
    return np.stack([res.results[b]["out"] for b in range(B)], axis=0)


# revision 28
# speedup vs baseline: 2.3261x; 1.0479x over previous
"""Trainium2 Bass kernel for nn_CrossAttention (B=8, Sq=Skv=2048, D=1024, C=768).

Strategy: data-parallel over batch — each of the 8 NeuronCores computes one
batch element's full cross-attention.

The projection chain is reassociated so every big contraction runs against
the NARROW context dim (C=768) instead of D=1024, and the K/V projections
disappear entirely (all exact identities, weights folded on host):

  scores = (x @ M + bqk) @ ctx^T          M   = Wq^T @ Wk   [D, C]
                                          bqk = bq @ Wk     [C]
  (bk drops: its score term is constant over k -> cancels in softmax)
  att    = softmax(scores / sqrt(D))
  final  = (e @ ctx)/sums @ WVO + bo''    WVO = (Wo @ Wv)^T [C, D]
                                          bo''= bo + Wo @ bv

FLOPs/core: 4.83 GMAC vs 16.1 GMAC for the naive pipeline.

Dtypes: bf16 operands everywhere (fp32 PSUM accumulation); the scores
matmul runs in fp8e4m3 with DoubleRow perf mode (2x PE throughput), with
xm/ctx held UNSCALED (sigma ~0.3-1, e4m3 normal range) and the 1/sqrt(D)
folded into the Exp activation. Measured end-to-end scale_rel ~1.2e-2
(tolerance 2e-2); set SCORES_FP8=False for a ~2e-3, slightly slower build.

Per-core phases:
  phase 1 (per 512-wide q chunk): xm^T[c,q] = M^T x^T + bqk, fp8 resident.
  phase 2 (per 512-wide q block):
    scores^T[k,q] accumulated over c (fp8 DoubleRow) -> exp -> expt bf16;
    DVE accumulates the softmax denominator across k-tiles; one fp32
    ones-matmul + e0-trick transpose -> 1/sums per q-partition.
    outp^T[c,q] = ctx_k^T @ expT accumulated over k.
    final[q,o] = outp^T.T @ WVO; evac fuses (*recip + bo'') in one DVE op.
"""

import numpy as np
import ml_dtypes

import concourse.bass as bass  # noqa: F401
import concourse.mybir as mybir
import concourse.tile as tile
from concourse import bacc
from concourse.bass_utils import run_bass_kernel_spmd

# ---- problem shapes (hardcoded) ----
B, SQ, SKV, D, C = 8, 2048, 2048, 1024, 768
P = 128
DT = D // P          # 8  d-tiles
CT = C // P          # 6  c-tiles
KT = SKV // P        # 16 k-tiles
QB = 512             # q block width
NQB = SQ // QB       # 4 q blocks
SCALE = 1.0 / np.sqrt(np.float32(D))

F32 = mybir.dt.float32
BF16 = mybir.dt.bfloat16
FP8 = mybir.dt.float8e4
AF = mybir.ActivationFunctionType
ALU = mybir.AluOpType
DR = mybir.MatmulPerfMode.DoubleRow

SCORES_FP8 = True

_NC_CACHE = {}


def build():
    if "nc" in _NC_CACHE:
        return _NC_CACHE["nc"]
    nc = bacc.Bacc(trn_type="TRN2", num_swdge_queues=4)

    KQDT = FP8 if SCORES_FP8 else BF16

    # ---- DRAM I/O (per-core slices; names = in_map keys) ----
    xT = nc.dram_tensor("xT", [D, SQ], BF16, kind="ExternalInput")
    ctx8T = nc.dram_tensor("ctx8T", [C, SKV], KQDT, kind="ExternalInput")
    ctxk = nc.dram_tensor("ctxk", [SKV, C], BF16, kind="ExternalInput")
    Mh = nc.dram_tensor("Mh", [D, C], BF16, kind="ExternalInput")
    wvoh = nc.dram_tensor("wvoh", [C, D], BF16, kind="ExternalInput")
    bqkh = nc.dram_tensor("bqkh", [P, CT], F32, kind="ExternalInput")
    bob = nc.dram_tensor("bob", [P, D], F32, kind="ExternalInput")
    onesmat = nc.dram_tensor("onesmat", [P, P], F32, kind="ExternalInput")
    e0two = nc.dram_tensor("e0two", [P, 2], F32, kind="ExternalInput")
    out = nc.dram_tensor("out", [SQ, D], F32, kind="ExternalOutput")

    with tile.TileContext(nc) as tc:
        with tc.tile_pool(name="persist", bufs=1) as persist:
            ctx8_sb = persist.tile([P, CT, SKV], KQDT, name="ctx8_sb")
            ctxk_sb = persist.tile([P, KT, C], BF16, name="ctxk_sb")   # 24KB/p
            xm_sb = persist.tile([P, CT, SQ], KQDT, name="xm_sb")
            m_sb = persist.tile([P, DT, C], BF16, name="m_sb")         # 12KB/p
            wvo_sb = persist.tile([P, CT, D], BF16, name="wvo_sb")     # 12KB/p
            bqk_sb = persist.tile([P, CT], F32, name="bqk_sb")
            bo_sb = persist.tile([P, D], F32, name="bo_sb")
            om_sb = persist.tile([P, P], F32, name="om_sb")
            e0_sb = persist.tile([P, 2], F32, name="e0_sb")
            sums_sb = persist.tile([P, QB], F32, name="sums_sb")

            with tc.tile_pool(name="p1_s", bufs=4) as p1_s:
                xt_tiles = [p1_s.tile([P, DT, QB], BF16, name="xt_sb",
                                      tag="xt") for qc in range(NQB)]
                # need-order DMA fanned across the three issue-capable
                # engines (sync/SP, scalar/ACT, gpsimd); first xm group's
                # operands go first
                # x and M gate phase 1 — they get all three engines' queue
                # groups first; ctx8/ctxk/wvo are needed only at ~50/65/80us
                # strict need-order round-robin over the three issue engines:
                # chunk-0's 16 pieces (M + xt0) split across all three queue
                # groups (~235GB/s aggregate) instead of riding one group
                engs = (nc.sync, nc.scalar, nc.gpsimd)
                ei = 0
                for it in range(DT):
                    engs[ei % 3].dma_start(m_sb[:, it],
                                           Mh[it * P:(it + 1) * P, :])
                    engs[(ei + 1) % 3].dma_start(xt_tiles[0][:, it],
                                                 xT[it * P:(it + 1) * P, 0:QB])
                    ei += 2
                nc.sync.dma_start(bqk_sb, bqkh[:])
                for qc in range(1, NQB):
                    for it in range(DT):
                        engs[ei % 3].dma_start(
                            xt_tiles[qc][:, it],
                            xT[it * P:(it + 1) * P, qc * QB:(qc + 1) * QB])
                        ei += 1
                nc.sync.dma_start(bo_sb, bob[:])
                nc.sync.dma_start(om_sb, onesmat[:])
                nc.sync.dma_start(e0_sb, e0two[:])

                # ===== phase 1: xm^T[c,q] = M^T @ x^T (+bqk), resident =====
                # ctx8/ctxk/wvo (6MB, needed at ~50/62/70us) are issued from
                # the scalar engine's stream AFTER per-chunk compute
                # milestones, so their transfers don't steal HBM bandwidth
                # from the x/M stream that gates this phase.
                with tc.tile_pool(name="ps_xm", bufs=3, space="PSUM") as ps_xm:
                    for qc in range(NQB):
                        for cs in range(CT):
                            pxm = ps_xm.tile([P, QB], F32, name="pxm", tag="pxm")
                            for it in range(DT):
                                nc.tensor.matmul(
                                    pxm, m_sb[:, it, cs * P:(cs + 1) * P],
                                    xt_tiles[qc][:, it],
                                    start=(it == 0), stop=(it == DT - 1))
                            nc.scalar.activation(
                                xm_sb[:, cs, qc * QB:(qc + 1) * QB], pxm,
                                AF.Identity, bias=bqk_sb[:, cs:cs + 1])
                        # ~5 issues per chunk keeps the scalar engine's evac
                        # cadence ahead of the PE
                        if qc == 0:
                            for t in range(CT):
                                nc.scalar.dma_start(ctx8_sb[:, t],
                                                    ctx8T[t * P:(t + 1) * P, :])
                        elif qc in (1, 2):
                            for kt_ in range((qc - 1) * 8, (qc - 1) * 8 + 8):
                                nc.scalar.dma_start(
                                    ctxk_sb[:, kt_],
                                    ctxk[kt_ * P:(kt_ + 1) * P, :])
                        else:
                            for t in range(CT):
                                nc.scalar.dma_start(wvo_sb[:, t],
                                                    wvoh[t * P:(t + 1) * P, :])

            # ================= phase 2: attention + fold-out ================
            with tc.tile_pool(name="p2_big", bufs=1) as p2_big, \
                 tc.tile_pool(name="p2_acc", bufs=3) as p2_acc, \
                 tc.tile_pool(name="p2_fin", bufs=4) as p2_fin, \
                 tc.tile_pool(name="p2_rcp", bufs=2) as p2_rcp, \
                 tc.tile_pool(name="ps_sc", bufs=2, space="PSUM") as ps_sc, \
                 tc.tile_pool(name="ps_po", bufs=2, space="PSUM") as ps_po, \
                 tc.tile_pool(name="ps_fin", bufs=2, space="PSUM") as ps_fin:
                expt_sb = p2_big.tile([P, KT, QB], BF16, name="expt_sb")
                outp_sb = p2_big.tile([P, CT, QB], BF16, name="outp_sb")
                for qb in range(NQB):
                    # ---- scores^T + exp; DVE accumulates denominator ----
                    # kt-tiles are processed in pairs sharing a 2-bank PSUM
                    # tile so ONE Exp covers 1024 columns: the ACT chain
                    # (~880ns/issue) stops pacing the PE's 650ns/group rate
                    acc = None
                    for kp in range(KT // 2):
                        psc = ps_sc.tile([P, 2, QB], F32, name="psc", tag="psc")
                        for j in range(2):
                            kt_ = kp * 2 + j
                            if SCORES_FP8:
                                for cs in range(0, CT, 2):
                                    nc.tensor.matmul(
                                        psc[:, j],
                                        ctx8_sb[:, cs:cs + 2, kt_ * P:(kt_ + 1) * P],
                                        xm_sb[:, cs:cs + 2, qb * QB:(qb + 1) * QB],
                                        start=(cs == 0), stop=(cs == CT - 2),
                                        perf_mode=DR)
                            else:
                                for cs in range(CT):
                                    nc.tensor.matmul(
                                        psc[:, j],
                                        ctx8_sb[:, cs, kt_ * P:(kt_ + 1) * P],
                                        xm_sb[:, cs, qb * QB:(qb + 1) * QB],
                                        start=(cs == 0), stop=(cs == CT - 1))
                        nc.scalar.activation(
                            expt_sb[:, 2 * kp:2 * kp + 2], psc, AF.Exp,
                            scale=float(SCALE) if SCORES_FP8 else 1.0)
                        pair = p2_acc.tile([P, QB], F32, name="pair", tag="acc")
                        nc.vector.tensor_add(pair, expt_sb[:, 2 * kp],
                                             expt_sb[:, 2 * kp + 1])
                        if kp == 0:
                            acc = pair
                        else:
                            nacc = p2_acc.tile([P, QB], F32, name="acc",
                                               tag="acc")
                            nc.vector.tensor_add(nacc, acc, pair)
                            acc = nacc
                    # ---- outp^T[c,q] = ctx_k^T @ expT, in cs pairs ----
                    def outp_pair(cp):
                        po0 = ps_po.tile([P, QB], F32, name="po0", tag="po")
                        po1 = ps_po.tile([P, QB], F32, name="po1", tag="po")
                        po = (po0, po1)
                        for kt_ in range(KT):
                            for cc in range(2):
                                c0 = (cp * 2 + cc) * P
                                nc.tensor.matmul(
                                    po[cc], ctxk_sb[:, kt_, c0:c0 + P],
                                    expt_sb[:, kt_],
                                    start=(kt_ == 0), stop=(kt_ == KT - 1))
                        for cc in range(2):
                            nc.scalar.copy(outp_sb[:, cp * 2 + cc], po[cc])
                    outp_pair(0)
                    # ---- sums: fp32 partition-reduce + e0-trick transpose ---
                    # psums/prt borrow the ps_fin pool: the final-projection
                    # groups only start ~15us later, so there's no overlap
                    psums = ps_fin.tile([P, QB], F32, name="psums", tag="pf")
                    nc.tensor.matmul(psums, om_sb, acc, start=True, stop=True)
                    nc.scalar.copy(sums_sb, psums)
                    prt = ps_fin.tile([P, 8], F32, name="prt", tag="pf")
                    for qs in range(4):
                        nc.tensor.matmul(
                            prt[:, 2 * qs:2 * qs + 2],
                            sums_sb[:, qs * P:(qs + 1) * P], e0_sb,
                            start=True, stop=True)
                    recip = p2_rcp.tile([P, 8], F32, name="recip", tag="recip")
                    nc.vector.reciprocal(recip, prt)
                    for cp in range(1, 3):
                        outp_pair(cp)
                    # ---- final = outp^T.T @ WVO; evac fuses *recip + bo'' ---
                    for qs in range(4):
                        for oc in range(2):
                            pf = ps_fin.tile([P, 512], F32, name="pf", tag="pf")
                            for cs in range(CT):
                                nc.tensor.matmul(
                                    pf, outp_sb[:, cs, qs * P:(qs + 1) * P],
                                    wvo_sb[:, cs, oc * 512:(oc + 1) * 512],
                                    start=(cs == 0), stop=(cs == CT - 1))
                            fin = p2_fin.tile([P, 512], F32, name="fin",
                                              tag="fin")
                            nc.vector.scalar_tensor_tensor(
                                fin, pf, recip[:, 2 * qs:2 * qs + 1],
                                bo_sb[:, oc * 512:(oc + 1) * 512],
                                op0=ALU.mult, op1=ALU.add)
                            seng = (nc.sync, nc.gpsimd, nc.scalar)[
                                (qs * 2 + oc) % 3]
                            seng.dma_start(
                                out[qb * QB + qs * P: qb * QB + (qs + 1) * P,
                                    oc * 512:(oc + 1) * 512], fin)
    nc.finalize()
    _NC_CACHE["nc"] = nc
    return nc


def _host_prep(x, context, Wq, bq, Wk, bk, Wv, bv, Wo, bo):
    """Build the 8 per-core input maps (host-side weight folding)."""
    BF = ml_dtypes.bfloat16
    F8np = ml_dtypes.float8_e4m3
    x = np.asarray(x, dtype=np.float32)
    context = np.asarray(context, dtype=np.float32)
    Wq64 = np.asarray(Wq, np.float64)
    Wk64 = np.asarray(Wk, np.float64)
    Wv64 = np.asarray(Wv, np.float64)
    Wo64 = np.asarray(Wo, np.float64)
    scale = np.float64(1.0) if SCORES_FP8 else np.float64(SCALE)
    M = (Wq64.T @ Wk64) * scale                       # [D, C]
    bqk = (np.asarray(bq, np.float64) @ Wk64) * scale  # [C]
    WVO = (Wo64 @ Wv64).T                             # [C, D]
    bo_eff = np.asarray(bo, np.float64) + Wo64 @ np.asarray(bv, np.float64)

    Mh = np.ascontiguousarray(M.astype(np.float32)).astype(BF)
    wvoh = np.ascontiguousarray(WVO.astype(np.float32)).astype(BF)
    bqkh = np.ascontiguousarray(
        bqk.astype(np.float32).reshape(CT, P).T)      # [p, ct]
    bob = np.ascontiguousarray(
        np.broadcast_to(bo_eff.astype(np.float32)[None, :], (P, D)))
    onesmat = np.ones((P, P), np.float32)
    e0two = np.zeros((P, 2), np.float32)
    e0two[0, :] = 1.0
    shared = dict(Mh=Mh, wvoh=wvoh, bqkh=bqkh, bob=bob,
                  onesmat=onesmat, e0two=e0two)
    xbf = x.astype(BF)
    cbf = context.astype(BF)
    in_maps = []
    for b in range(B):
        m = dict(shared)
        m["xT"] = np.ascontiguousarray(xbf[b].T)              # [D, SQ] bf16
        ctxTb = np.ascontiguousarray(cbf[b].T)                # [C, SKV]
        m["ctx8T"] = ctxTb.astype(F8np) if SCORES_FP8 else ctxTb
        m["ctxk"] = np.ascontiguousarray(cbf[b])              # [SKV, C] bf16
        in_maps.append(m)
    return in_maps


def kernel(**inputs) -> np.ndarray:
    nc = build()
    in_maps = _host_prep(**inputs)
    res = run_bass_kernel_spmd(nc, in_maps, core_ids=list(range(B)))
    return np.stack([res.results[b]["out"] for b in range(B)], axis=0)


# revision 29
# speedup vs baseline: 2.3833x; 1.0246x over previous
"""Trainium2 Bass kernel for nn_CrossAttention (B=8, Sq=Skv=2048, D=1024, C=768).

Strategy: data-parallel over batch — each of the 8 NeuronCores computes one
batch element's full cross-attention.

The projection chain is reassociated so every big contraction runs against
the NARROW context dim (C=768) instead of D=1024, and the K/V projections
disappear entirely (all exact identities, weights folded on host):

  scores = (x @ M + bqk) @ ctx^T          M   = Wq^T @ Wk   [D, C]
                                          bqk = bq @ Wk     [C]
  (bk drops: its score term is constant over k -> cancels in softmax)
  att    = softmax(scores / sqrt(D))
  final  = (e @ ctx)/sums @ WVO + bo''    WVO = (Wo @ Wv)^T [C, D]
                                          bo''= bo + Wo @ bv

FLOPs/core: 4.83 GMAC vs 16.1 GMAC for the naive pipeline.

Dtypes: bf16 operands everywhere (fp32 PSUM accumulation); the scores
matmul runs in fp8e4m3 with DoubleRow perf mode (2x PE throughput), with
xm/ctx held UNSCALED (sigma ~0.3-1, e4m3 normal range) and the 1/sqrt(D)
folded into the Exp activation. Measured end-to-end scale_rel ~1.2e-2
(tolerance 2e-2); set SCORES_FP8=False for a ~2e-3, slightly slower build.

Per-core phases:
  phase 1 (per 512-wide q chunk): xm^T[c,q] = M^T x^T + bqk, fp8 resident.
  phase 2 (per 512-wide q block):
    scores^T[k,q] accumulated over c (fp8 DoubleRow) -> exp -> expt bf16;
    DVE accumulates the softmax denominator across k-tiles; one fp32
    ones-matmul + e0-trick transpose -> 1/sums per q-partition.
    outp^T[c,q] = ctx_k^T @ expT accumulated over k.
    final[q,o] = outp^T.T @ WVO; evac fuses (*recip + bo'') in one DVE op.
"""

import numpy as np
import ml_dtypes

import concourse.bass as bass  # noqa: F401
import concourse.mybir as mybir
import concourse.tile as tile
from concourse import bacc
from concourse.bass_utils import run_bass_kernel_spmd

# ---- problem shapes (hardcoded) ----
B, SQ, SKV, D, C = 8, 2048, 2048, 1024, 768
P = 128
DT = D // P          # 8  d-tiles
CT = C // P          # 6  c-tiles
KT = SKV // P        # 16 k-tiles
QB = 512             # q block width
NQB = SQ // QB       # 4 q blocks
SCALE = 1.0 / np.sqrt(np.float32(D))

F32 = mybir.dt.float32
BF16 = mybir.dt.bfloat16
FP8 = mybir.dt.float8e4
AF = mybir.ActivationFunctionType
ALU = mybir.AluOpType
DR = mybir.MatmulPerfMode.DoubleRow

SCORES_FP8 = True

_NC_CACHE = {}


def build():
    if "nc" in _NC_CACHE:
        return _NC_CACHE["nc"]
    nc = bacc.Bacc(trn_type="TRN2", num_swdge_queues=4)

    KQDT = FP8 if SCORES_FP8 else BF16

    # ---- DRAM I/O (per-core slices; names = in_map keys) ----
    xT = nc.dram_tensor("xT", [D, SQ], BF16, kind="ExternalInput")
    ctx8T = nc.dram_tensor("ctx8T", [C, SKV], KQDT, kind="ExternalInput")
    ctxk = nc.dram_tensor("ctxk", [SKV, C], BF16, kind="ExternalInput")
    Mh = nc.dram_tensor("Mh", [D, C], BF16, kind="ExternalInput")
    wvoh = nc.dram_tensor("wvoh", [C, D], BF16, kind="ExternalInput")
    bqkh = nc.dram_tensor("bqkh", [P, CT], F32, kind="ExternalInput")
    bob = nc.dram_tensor("bob", [P, D], F32, kind="ExternalInput")
    onesmat = nc.dram_tensor("onesmat", [P, P], BF16, kind="ExternalInput")
    e0two = nc.dram_tensor("e0two", [P, 2], BF16, kind="ExternalInput")
    out = nc.dram_tensor("out", [SQ, D], F32, kind="ExternalOutput")

    with tile.TileContext(nc) as tc:
        with tc.tile_pool(name="persist", bufs=1) as persist:
            ctx8_sb = persist.tile([P, CT, SKV], KQDT, name="ctx8_sb")
            ctxk_sb = persist.tile([P, KT, C], BF16, name="ctxk_sb")   # 24KB/p
            xm_sb = persist.tile([P, CT, SQ], KQDT, name="xm_sb")
            m_sb = persist.tile([P, DT, C], BF16, name="m_sb")         # 12KB/p
            wvo_sb = persist.tile([P, CT, D], BF16, name="wvo_sb")     # 12KB/p
            bqk_sb = persist.tile([P, CT], F32, name="bqk_sb")
            bo_sb = persist.tile([P, D], F32, name="bo_sb")
            om_sb = persist.tile([P, P], BF16, name="om_sb")
            e0_sb = persist.tile([P, 2], BF16, name="e0_sb")
            sums_sb = persist.tile([P, QB], BF16, name="sums_sb")

            with tc.tile_pool(name="p1_s", bufs=4) as p1_s:
                xt_tiles = [p1_s.tile([P, DT, QB], BF16, name="xt_sb",
                                      tag="xt") for qc in range(NQB)]
                # need-order DMA fanned across the three issue-capable
                # engines (sync/SP, scalar/ACT, gpsimd); first xm group's
                # operands go first
                # x and M gate phase 1 — they get all three engines' queue
                # groups first; ctx8/ctxk/wvo are needed only at ~50/65/80us
                # strict need-order round-robin over the three issue engines:
                # chunk-0's 16 pieces (M + xt0) split across all three queue
                # groups (~235GB/s aggregate) instead of riding one group
                engs = (nc.sync, nc.scalar, nc.gpsimd)
                ei = 0
                for it in range(DT):
                    engs[ei % 3].dma_start(m_sb[:, it],
                                           Mh[it * P:(it + 1) * P, :])
                    engs[(ei + 1) % 3].dma_start(xt_tiles[0][:, it],
                                                 xT[it * P:(it + 1) * P, 0:QB])
                    ei += 2
                nc.sync.dma_start(bqk_sb, bqkh[:])
                for qc in range(1, NQB):
                    for it in range(DT):
                        engs[ei % 3].dma_start(
                            xt_tiles[qc][:, it],
                            xT[it * P:(it + 1) * P, qc * QB:(qc + 1) * QB])
                        ei += 1
                nc.sync.dma_start(bo_sb, bob[:])
                nc.sync.dma_start(om_sb, onesmat[:])
                nc.sync.dma_start(e0_sb, e0two[:])

                # ===== phase 1: xm^T[c,q] = M^T @ x^T (+bqk), resident =====
                # ctx8/ctxk/wvo (6MB, needed at ~50/62/70us) are issued from
                # the scalar engine's stream AFTER per-chunk compute
                # milestones, so their transfers don't steal HBM bandwidth
                # from the x/M stream that gates this phase.
                with tc.tile_pool(name="ps_xm", bufs=3, space="PSUM") as ps_xm:
                    for qc in range(NQB):
                        for cs in range(CT):
                            pxm = ps_xm.tile([P, QB], F32, name="pxm", tag="pxm")
                            for it in range(DT):
                                nc.tensor.matmul(
                                    pxm, m_sb[:, it, cs * P:(cs + 1) * P],
                                    xt_tiles[qc][:, it],
                                    start=(it == 0), stop=(it == DT - 1))
                            nc.scalar.activation(
                                xm_sb[:, cs, qc * QB:(qc + 1) * QB], pxm,
                                AF.Identity, bias=bqk_sb[:, cs:cs + 1])
                        # ~5 issues per chunk keeps the scalar engine's evac
                        # cadence ahead of the PE
                        if qc == 0:
                            for t in range(CT):
                                nc.scalar.dma_start(ctx8_sb[:, t],
                                                    ctx8T[t * P:(t + 1) * P, :])
                        elif qc in (1, 2):
                            for kt_ in range((qc - 1) * 8, (qc - 1) * 8 + 8):
                                nc.scalar.dma_start(
                                    ctxk_sb[:, kt_],
                                    ctxk[kt_ * P:(kt_ + 1) * P, :])
                        else:
                            for t in range(CT):
                                nc.scalar.dma_start(wvo_sb[:, t],
                                                    wvoh[t * P:(t + 1) * P, :])

            # ================= phase 2: attention + fold-out ================
            with tc.tile_pool(name="p2_big", bufs=1) as p2_big, \
                 tc.tile_pool(name="p2_acc", bufs=3) as p2_acc, \
                 tc.tile_pool(name="p2_fin", bufs=4) as p2_fin, \
                 tc.tile_pool(name="p2_rcp", bufs=2) as p2_rcp, \
                 tc.tile_pool(name="ps_sc", bufs=2, space="PSUM") as ps_sc, \
                 tc.tile_pool(name="ps_po", bufs=2, space="PSUM") as ps_po, \
                 tc.tile_pool(name="ps_fin", bufs=2, space="PSUM") as ps_fin:
                expt_sb = p2_big.tile([P, KT, QB], BF16, name="expt_sb")
                outp_sb = p2_big.tile([P, CT, QB], BF16, name="outp_sb")
                for qb in range(NQB):
                    # ---- scores^T + exp; DVE accumulates denominator ----
                    # kt-tiles are processed in pairs sharing a 2-bank PSUM
                    # tile so ONE Exp covers 1024 columns: the ACT chain
                    # (~880ns/issue) stops pacing the PE's 650ns/group rate
                    acc = None
                    for kp in range(KT // 2):
                        psc = ps_sc.tile([P, 2, QB], F32, name="psc", tag="psc")
                        for j in range(2):
                            kt_ = kp * 2 + j
                            if SCORES_FP8:
                                for cs in range(0, CT, 2):
                                    nc.tensor.matmul(
                                        psc[:, j],
                                        ctx8_sb[:, cs:cs + 2, kt_ * P:(kt_ + 1) * P],
                                        xm_sb[:, cs:cs + 2, qb * QB:(qb + 1) * QB],
                                        start=(cs == 0), stop=(cs == CT - 2),
                                        perf_mode=DR)
                            else:
                                for cs in range(CT):
                                    nc.tensor.matmul(
                                        psc[:, j],
                                        ctx8_sb[:, cs, kt_ * P:(kt_ + 1) * P],
                                        xm_sb[:, cs, qb * QB:(qb + 1) * QB],
                                        start=(cs == 0), stop=(cs == CT - 1))
                        nc.scalar.activation(
                            expt_sb[:, 2 * kp:2 * kp + 2], psc, AF.Exp,
                            scale=float(SCALE) if SCORES_FP8 else 1.0)
                        pair = p2_acc.tile([P, QB], BF16, name="pair", tag="acc")
                        nc.vector.tensor_add(pair, expt_sb[:, 2 * kp],
                                             expt_sb[:, 2 * kp + 1])
                        if kp == 0:
                            acc = pair
                        else:
                            nacc = p2_acc.tile([P, QB], BF16, name="acc",
                                               tag="acc")
                            nc.vector.tensor_add(nacc, acc, pair)
                            acc = nacc
                    # ---- outp^T[c,q] = ctx_k^T @ expT, in cs pairs ----
                    def outp_pair(cp):
                        po0 = ps_po.tile([P, QB], F32, name="po0", tag="po")
                        po1 = ps_po.tile([P, QB], F32, name="po1", tag="po")
                        po = (po0, po1)
                        for kt_ in range(KT):
                            for cc in range(2):
                                c0 = (cp * 2 + cc) * P
                                nc.tensor.matmul(
                                    po[cc], ctxk_sb[:, kt_, c0:c0 + P],
                                    expt_sb[:, kt_],
                                    start=(kt_ == 0), stop=(kt_ == KT - 1))
                        for cc in range(2):
                            nc.scalar.copy(outp_sb[:, cp * 2 + cc], po[cc])
                    outp_pair(0)
                    # ---- sums: fp32 partition-reduce + e0-trick transpose ---
                    # psums/prt borrow the ps_fin pool: the final-projection
                    # groups only start ~15us later, so there's no overlap
                    psums = ps_fin.tile([P, QB], F32, name="psums", tag="pf")
                    nc.tensor.matmul(psums, om_sb, acc, start=True, stop=True)
                    nc.scalar.copy(sums_sb, psums)
                    prt = ps_fin.tile([P, 8], F32, name="prt", tag="pf")
                    for qs in range(4):
                        nc.tensor.matmul(
                            prt[:, 2 * qs:2 * qs + 2],
                            sums_sb[:, qs * P:(qs + 1) * P], e0_sb,
                            start=True, stop=True)
                    recip = p2_rcp.tile([P, 8], F32, name="recip", tag="recip")
                    nc.vector.reciprocal(recip, prt)
                    for cp in range(1, 3):
                        outp_pair(cp)
                    # ---- final = outp^T.T @ WVO; evac fuses *recip + bo'' ---
                    for qs in range(4):
                        for oc in range(2):
                            pf = ps_fin.tile([P, 512], F32, name="pf", tag="pf")
                            for cs in range(CT):
                                nc.tensor.matmul(
                                    pf, outp_sb[:, cs, qs * P:(qs + 1) * P],
                                    wvo_sb[:, cs, oc * 512:(oc + 1) * 512],
                                    start=(cs == 0), stop=(cs == CT - 1))
                            fin = p2_fin.tile([P, 512], F32, name="fin",
                                              tag="fin")
                            nc.vector.scalar_tensor_tensor(
                                fin, pf, recip[:, 2 * qs:2 * qs + 1],
                                bo_sb[:, oc * 512:(oc + 1) * 512],
                                op0=ALU.mult, op1=ALU.add)
                            seng = (nc.sync, nc.gpsimd, nc.scalar)[
                                (qs * 2 + oc) % 3]
                            seng.dma_start(
                                out[qb * QB + qs * P: qb * QB + (qs + 1) * P,
                                    oc * 512:(oc + 1) * 512], fin)
    nc.finalize()
    _NC_CACHE["nc"] = nc
    return nc


def _host_prep(x, context, Wq, bq, Wk, bk, Wv, bv, Wo, bo):
    """Build the 8 per-core input maps (host-side weight folding)."""
    BF = ml_dtypes.bfloat16
    F8np = ml_dtypes.float8_e4m3
    x = np.asarray(x, dtype=np.float32)
    context = np.asarray(context, dtype=np.float32)
    Wq64 = np.asarray(Wq, np.float64)
    Wk64 = np.asarray(Wk, np.float64)
    Wv64 = np.asarray(Wv, np.float64)
    Wo64 = np.asarray(Wo, np.float64)
    scale = np.float64(1.0) if SCORES_FP8 else np.float64(SCALE)
    M = (Wq64.T @ Wk64) * scale                       # [D, C]
    bqk = (np.asarray(bq, np.float64) @ Wk64) * scale  # [C]
    WVO = (Wo64 @ Wv64).T                             # [C, D]
    bo_eff = np.asarray(bo, np.float64) + Wo64 @ np.asarray(bv, np.float64)

    Mh = np.ascontiguousarray(M.astype(np.float32)).astype(BF)
    wvoh = np.ascontiguousarray(WVO.astype(np.float32)).astype(BF)
    bqkh = np.ascontiguousarray(
        bqk.astype(np.float32).reshape(CT, P).T)      # [p, ct]
    bob = np.ascontiguousarray(
        np.broadcast_to(bo_eff.astype(np.float32)[None, :], (P, D)))
    onesmat = np.ones((P, P), np.float32).astype(BF)
    e0two = np.zeros((P, 2), np.float32)
    e0two[0, :] = 1.0
    e0two = e0two.astype(BF)
    shared = dict(Mh=Mh, wvoh=wvoh, bqkh=bqkh, bob=bob,
                  onesmat=onesmat, e0two=e0two)
    xbf = x.astype(BF)
    cbf = context.astype(BF)
    in_maps = []
    for b in range(B):
        m = dict(shared)
        m["xT"] = np.ascontiguousarray(xbf[b].T)              # [D, SQ] bf16
        ctxTb = np.ascontiguousarray(cbf[b].T)                # [C, SKV]
        m["ctx8T"] = ctxTb.astype(F8np) if SCORES_FP8 else ctxTb
        m["ctxk"] = np.ascontiguousarray(cbf[b])              # [SKV, C] bf16
        in_maps.append(m)
    return in_maps


def kernel(**inputs) -> np.ndarray:
    nc = build()
    in_maps = _host_prep(**inputs)
    res = run_bass_kernel_spmd(nc, in_maps, core_ids=list(range(B)))
    return np.stack([res.results[b]["out"] for b in range(B)], axis=0)
